# revision 38
# baseline (speedup 1.0000x reference)
"""Tensor-parallel causal attention kernel for 8 trn2 NeuronCores.

Problem: B=2, S=2048, H=2048, 16 heads, head_dim=128 fp32.
  qkv = hidden @ w_qkv.T ; causal attention ; out = attn @ w_o.T

Sharding (hardcoded): core c in 0..7 handles batch b=c//4 and heads
hs = [4*(c%4) .. 4*(c%4)+3].  Each core computes a partial o_proj
output (contraction over its 512 hidden dims); the host sums the 4
partials per batch and transposes.  No device collectives.

Device-side layout (all host-pretiled to partition-major [128, ...]):
  xt  [128,16,2048] f32 : xt[p,ko,s]  = hidden[b, s, ko*128+p]
  wq  [128,16, 512] f32 : wq[p,ko,o]  = w_qkv[q_rows[o],  ko*128+p]
  wk  [128,16, 512] f32 : wk[p,ko,o]  = w_qkv[k_rows[o],  ko*128+p]
  wv  [128,16, 512] f32 : wv[p,ko,d]  = w_qkv[v_rows[d],  ko*128+p]
  wo  [128, 4,2048] f32 : wo[p,kb,o]  = w_o[o, cols[kb*128+p]]
  outt[128,16,2048] f32 : outt[p,ot,s] = outT_partial[ot*128+p, s]

Design (single interleaved pipeline over four 512-token chunks):
  p1(0), p2(0), p1(1), p3(0), p2(1), p1(2), p3(1), ... p2(3), p3(3)
  - p1 projects one x chunk (Q,K columns + V blocks); psum->sbuf casts
    run on the scalar engine (idle during p1), prologue DMAs are
    per-ko-striped so the first V group trickles behind the DMA wave.
  - p2 is software-pipelined causal attention: sum+attnV consumers
    trail scores+exp by two k-pairs; diagonal blocks are processed
    first and stream only their valid q range (block-128-granular
    causality, bit-identical results); softmax sums use a [P,16] ones
    stationary so the PSUM drain spreads across 16 partitions.
  - p3 o_proj partials stage through bf16 sbuf tiles (10 bufs) and DMA
    out in bf16; the host accumulates the 4 partials per batch in f32.

Toolchain quirk workarounds (walrus 1-sync-wait slot):
  - chunked tail drain monkeypatch (single sem-only barrier, no
    trailing barrier)
  - post-pass hoisting extra sem waits onto same-engine NoOps
"""
import numpy as np

import concourse.bass as bass
import concourse.mybir as mybir
import concourse.tile as tile
from concourse.bass_utils import run_bass_kernel_spmd
from concourse.vector_clock import ScopedClock, VectorClock

P = 128
S = 2048
H = 2048
NH_LOCAL = 4          # heads per core
KO = H // P           # 16 contraction chunks for the projections
SQ = 512              # q chunk width
NQC = S // SQ         # 4 q chunks
NKB = S // P          # 16 key blocks
F32 = mybir.dt.float32
BF = mybir.dt.bfloat16
AF = mybir.ActivationFunctionType
SCALE = 1.0 / float(np.sqrt(128.0))

XCH = SQ              # x chunk width == q chunk width (interleaved pipe)
NXCH = S // XCH


def _drain_and_barrier_chunked(self, tick_clock, wait_clock, _MAX=1):
    """Split the kernel-tail drain's waits: walrus allows only one sync
    wait per instruction in this toolchain."""
    g = tick_clock.global_clock
    n = len(g)
    vals = [g[i] for i in range(n)]
    nz = [i for i, v in enumerate(vals) if v > 0]
    chunks = [nz[i:i + _MAX] for i in range(0, len(nz), _MAX)] or [[]]
    for chunk in chunks:
        vec = [vals[i] if i in chunk else 0 for i in range(n)]
        d = self.nc.sync.drain()
        wait_clock.add_sem_waits(d.ins, ScopedClock({None: VectorClock(vec)}))
    self.nc.all_engine_barrier(sem_only=True)
    assert self.sems is not None
    popped = self.nc._tile_sem_poison_stack.pop()
    assert popped is self._sem_poison
    # no trailing barrier: engines are already synced by the barrier
    # above, and the sem clear below is the true last operation.
    self.nc.clear_and_free_semaphores(list(self.sems.allocated().values()))


tile.TileContext._drain_and_barrier = _drain_and_barrier_chunked


def _split_multi_waits(nc):
    """walrus allows ONE sync wait per instruction: hoist extra waits onto
    same-engine NoOps inserted directly before the offending instruction
    (identical semantics — the engine queue blocks on each in turn)."""
    ctr = 0
    for f in nc.m.functions:
        for blk in f.blocks:
            new = []
            changed = False
            for inst in blk.instructions:
                si = inst.sync_info
                waits = list(si.on_wait) if si and si.on_wait else []
                if len(waits) > 1:
                    changed = True
                    for w in waits[:-1]:
                        ctr += 1
                        nop = mybir.InstNoOp(name=f"I-wsplit-{ctr}",
                                             engine=inst.engine,
                                             ins=[], outs=[])
                        nop.sync_info = mybir.SyncInfo(on_wait=[w],
                                                       on_update=[])
                        new.append(nop)
                    ups = list(si.on_update) if si.on_update else []
                    inst.sync_info = mybir.SyncInfo(on_wait=[waits[-1]],
                                                    on_update=ups)
                new.append(inst)
            if changed:
                blk.instructions = new
    return ctr


def build():
    nc = bass.Bass()
    xt = nc.dram_tensor("xt", [P, KO, S], BF, kind="ExternalInput")
    wq = nc.dram_tensor("wq", [P, KO, NH_LOCAL * P], BF, kind="ExternalInput")
    wk = nc.dram_tensor("wk", [P, KO, NH_LOCAL * P], BF, kind="ExternalInput")
    wv = nc.dram_tensor("wv", [P, KO, NH_LOCAL * P], BF, kind="ExternalInput")
    wo = nc.dram_tensor("wo", [P, NH_LOCAL, S], BF, kind="ExternalInput")
    outt = nc.dram_tensor("outt", [P, KO, S], BF, kind="ExternalOutput")

    with tile.TileContext(nc) as tc:
        from contextlib import ExitStack
        with ExitStack() as ctx:
            const = ctx.enter_context(tc.tile_pool(name="const", bufs=1))
            ones16_bf = const.tile([P, 16], BF)
            nc.vector.memset(ones16_bf[:], 1.0)
            onesrow_bf = const.tile([1, P], BF)
            nc.vector.memset(onesrow_bf[:], 1.0)

            # ---- resident SBUF tensors --------------------------------
            res = ctx.enter_context(tc.tile_pool(name="res", bufs=1))
            # Q,K as qkvT: [d_in, ot(0-3 Q heads, 4-7 K heads), s]
            qk_sb = res.tile([P, 2 * NH_LOCAL, S], BF)
            # V as [s_in, kb, d_local]
            v_sb = res.tile([P, NKB, NH_LOCAL * P], BF)
            attnT = res.tile([P, NH_LOCAL, S], BF)
            w_r = res.tile([P, KO, 3 * NH_LOCAL * P], BF)   # Q|K|V weights
            wo_r = res.tile([P, NH_LOCAL, S], BF)

            xp = ctx.enter_context(tc.tile_pool(name="xp", bufs=2))
            estp = ctx.enter_context(tc.tile_pool(name="estp", bufs=6))
            smallp = ctx.enter_context(tc.tile_pool(name="smallp", bufs=2))
            stagep = ctx.enter_context(tc.tile_pool(name="stagep", bufs=10))

            ps_big = ctx.enter_context(
                tc.tile_pool(name="ps_big", bufs=2, space="PSUM"))
            ps_sm = ctx.enter_context(
                tc.tile_pool(name="ps_sm", bufs=2, space="PSUM"))
            ps_sum = ctx.enter_context(
                tc.tile_pool(name="ps_sum", bufs=2, space="PSUM"))

            # ---- prologue DMAs ----------------------------------------
            # x0 + wv striped per-ko so the first V group's k-loop can
            # trickle behind the DMA wavefront (dma queues round-robin;
            # strip k lands before strip k+8).  wq follows in quarters so
            # the Q groups can start right after the V groups drain.
            x_tiles = {}
            x0 = xp.tile([P, KO, XCH], BF, tag="xr", name="xr0")
            # single-ko strips for the first 4 ko (earliest possible
            # first matmul), then 2-ko strips (2KB dma lines)
            strip_slices = ([slice(k, k + 1) for k in range(4)]
                            + [slice(k, k + 2) for k in range(4, KO, 2)])
            for ks in strip_slices:
                nc.sync.dma_start(x0[:, ks], xt.ap()[:, ks, 0:XCH])
                nc.sync.dma_start(
                    w_r[:, ks, 2 * NH_LOCAL * P:3 * NH_LOCAL * P],
                    wv.ap()[:, ks])
            x_tiles[0] = x0
            for kq in range(4):
                ks = slice(4 * kq, 4 * (kq + 1))
                nc.sync.dma_start(w_r[:, ks, 0:NH_LOCAL * P],
                                  wq.ap()[:, ks])
            for kq in range(4):
                ks = slice(4 * kq, 4 * (kq + 1))
                nc.sync.dma_start(
                    w_r[:, ks, NH_LOCAL * P:2 * NH_LOCAL * P],
                    wk.ap()[:, ks])
            nc.sync.dma_start(wo_r[:], wo.ap())

            # ---- phase helpers ----------------------------------------
            def p1_chunk(xc):
                """Project x chunk xc -> Q,K chunk cols + V blocks.
                PSUM->SBUF casts go to the scalar engine (idle here),
                keeping the DVE queue clear for p2/p3 work."""
                if xc + 1 < NXCH:     # prefetch next x chunk (4 queues)
                    x_n = xp.tile([P, KO, XCH], BF, tag="xr")
                    for kq in range(4):
                        ks = slice(4 * kq, 4 * (kq + 1))
                        nc.sync.dma_start(
                            x_n[:, ks],
                            xt.ap()[:, ks, (xc + 1) * XCH:(xc + 2) * XCH])
                    x_tiles[xc + 1] = x_n
                x_r = x_tiles[xc]
                # V first (st blocks): out [s=128, d=512]
                for st in range(XCH // P):
                    kb = xc * (XCH // P) + st
                    ps = ps_big.tile([P, 2 * SQ], F32, tag="big")
                    for k in range(KO):
                        nc.tensor.matmul(
                            ps[:, 0:NH_LOCAL * P],
                            x_r[:, k, st * P:(st + 1) * P],
                            w_r[:, k, 2 * NH_LOCAL * P:3 * NH_LOCAL * P],
                            start=(k == 0), stop=(k == KO - 1))
                    nc.scalar.copy(v_sb[:, kb, :], ps[:, 0:NH_LOCAL * P])
                # Q then K o-tiles: out [o=128, s=XCH]
                for ot in range(2 * NH_LOCAL):
                    ps = ps_big.tile([P, 2 * SQ], F32, tag="big")
                    for k in range(KO):
                        nc.tensor.matmul(
                            ps[:, 0:XCH], w_r[:, k, ot * P:(ot + 1) * P],
                            x_r[:, k], start=(k == 0), stop=(k == KO - 1))
                    nc.scalar.copy(
                        qk_sb[:, ot, xc * XCH:(xc + 1) * XCH], ps[:, 0:XCH])

            def p2_chunk(qc):
                """Causal attention for q chunk qc, all local heads.
                Software-pipelined: consumers (sum+attnV) of pair g are
                emitted two pairs behind scores+exp.  Diagonal blocks are
                processed FIRST so the end-of-head flush lands on pairs
                whose est needs no gpsimd mask; they also stream only
                their valid q range [off, SQ) — the skipped columns are
                exactly the all-masked (zero) ones.  The first processed
                block (kb=4*qc) has off==0, so the start=True matmuls
                initialize every column."""
                nkb = 4 * (qc + 1)
                npair = nkb // 2
                qs = qc * SQ
                # processed block order: 4 diagonal blocks, then the rest
                blocks = list(range(nkb - 4, nkb)) + list(range(0, nkb - 4))
                state = {}            # h -> (at_ps, sm_ps)
                pending = []          # [(h, pi, est)], consumers lag 2

                def consumers(h, pi, est):
                    at_ps, sm_ps = state[h]
                    # both sums back-to-back: one ones-weights load
                    for j in range(2):
                        pos = 2 * pi + j
                        kb = blocks[pos]
                        off = max(0, kb * P - qs)
                        nc.tensor.matmul(sm_ps[0:16, off:SQ], ones16_bf[:],
                                         est[:, j * SQ + off:(j + 1) * SQ],
                                         start=(pos == 0),
                                         stop=(pos == nkb - 1))
                    for j in range(2):
                        pos = 2 * pi + j
                        kb = blocks[pos]
                        off = max(0, kb * P - qs)
                        nc.tensor.matmul(
                            at_ps[:, off:SQ],
                            v_sb[:, kb, h * P:(h + 1) * P],
                            est[:, j * SQ + off:(j + 1) * SQ],
                            start=(pos == 0), stop=(pos == nkb - 1))
                    if pi == npair - 1:   # head tail
                        logs = smallp.tile([1, SQ], F32, tag="logs")
                        nc.scalar.activation(logs[:], sm_ps[0:1, :], AF.Ln)
                        recip = smallp.tile([1, SQ], BF, tag="recip")
                        nc.scalar.activation(recip[:], logs[:], AF.Exp,
                                             scale=-1.0)
                        # broadcast recip to 128 partitions via K=1 matmul
                        # (reuses the sum-tag psum slot; Ln has already
                        # read the old contents by the time this runs)
                        rep_ps = ps_sum.tile([P, SQ], F32, tag="sum",
                                             name=f"rep{qc}_{h}")
                        nc.tensor.matmul(rep_ps[:], onesrow_bf[:], recip[:],
                                         start=True, stop=True)
                        rep_sb = smallp.tile([P, SQ], F32, tag="rep")
                        nc.vector.tensor_copy(rep_sb[:], rep_ps[:])
                        nc.vector.tensor_mul(attnT[:, h, qs:qs + SQ],
                                             at_ps[:], rep_sb[:])
                        del state[h]

                for h in range(NH_LOCAL):
                    state[h] = (ps_sm.tile([P, SQ], F32, tag="small", name=f"at{qc}_{h}"),
                                ps_sum.tile([P, SQ], F32, tag="sum", name=f"sm{qc}_{h}"))
                    for pi in range(npair):
                        b0, b1 = blocks[2 * pi], blocks[2 * pi + 1]
                        st_ps = ps_big.tile([P, 2 * SQ], F32, tag="big")
                        for j, kb in enumerate((b0, b1)):
                            off = max(0, kb * P - qs)
                            nc.tensor.matmul(
                                st_ps[:, j * SQ + off:(j + 1) * SQ],
                                qk_sb[:, NH_LOCAL + h, kb * P:(kb + 1) * P],
                                qk_sb[:, h, qs + off:qs + SQ],
                                start=True, stop=True)
                        est = estp.tile([P, 2 * SQ], BF, tag="est")
                        off0 = max(0, b0 * P - qs)
                        # [SQ, SQ+off1) holds never-written garbage; its
                        # exp output is never read (consumers start at
                        # off1 within block j=1).
                        nc.scalar.activation(est[:, off0:2 * SQ],
                                             st_ps[:, off0:2 * SQ], AF.Exp,
                                             scale=SCALE)
                        for j, kb in enumerate((b0, b1)):
                            off = max(0, kb * P - qs)
                            if kb * P + P - 1 > qs:  # crosses diagonal
                                nc.gpsimd.affine_select(
                                    est[:, j * SQ + off:(j + 1) * SQ],
                                    est[:, j * SQ + off:(j + 1) * SQ],
                                    [[1, SQ - off]],
                                    mybir.AluOpType.is_ge, 0.0,
                                    base=qs + off - kb * P,
                                    channel_multiplier=-1)
                        if len(pending) >= 2:
                            consumers(*pending.pop(0))
                        pending.append((h, pi, est))
                while pending:
                    consumers(*pending.pop(0))

            def p3_chunk(sc):
                """o_proj partial for q chunk sc."""
                for ot in range(KO):
                    ps = ps_sm.tile([P, SQ], F32, tag="small")
                    for kb in range(NH_LOCAL):
                        nc.tensor.matmul(
                            ps[:], wo_r[:, kb, ot * P:(ot + 1) * P],
                            attnT[:, kb, sc * SQ:(sc + 1) * SQ],
                            start=(kb == 0), stop=(kb == NH_LOCAL - 1))
                    stage = stagep.tile([P, SQ], BF, tag="p3stage")
                    nc.vector.tensor_copy(stage[:], ps[:])
                    nc.sync.dma_start(
                        outt.ap()[:, ot, sc * SQ:(sc + 1) * SQ], stage[:])

            # ---- interleaved pipeline ---------------------------------
            # p1(1) is hoisted before p2(0): the first attention chunk
            # has only 2 k-pairs per head (pipeline can't fill), so it
            # runs with 41us of projection matmuls ahead of it to hide
            # its exp/mask latency.
            p1_chunk(0)
            p1_chunk(1)
            p2_chunk(0)
            p3_chunk(0)
            for xc in range(1, NXCH):
                p2_chunk(xc)
                if xc + 1 < NXCH:
                    p1_chunk(xc + 1)
                p3_chunk(xc)
    _split_multi_waits(nc)
    return nc


_NC_CACHE = None


def _get_nc():
    global _NC_CACHE
    if _NC_CACHE is None:
        _NC_CACHE = build()
    return _NC_CACHE


def _prep_inputs(hidden_states, w_qkv, w_o):
    """Host-side shard + pre-tile + bf16-cast for the 8 cores."""
    import ml_dtypes
    BF_NP = ml_dtypes.bfloat16
    hidden_states = np.asarray(hidden_states, dtype=np.float32)
    w_qkv = np.asarray(w_qkv, dtype=np.float32)
    w_o = np.asarray(w_o, dtype=np.float32)
    B = hidden_states.shape[0]

    in_maps = []
    xt_by_b = {}
    for b in range(B):
        # xt[p, ko, s] = hidden[b, s, ko*128+p]
        xt = np.ascontiguousarray(
            hidden_states[b].T.reshape(KO, P, S).transpose(1, 0, 2)
        ).astype(BF_NP)
        xt_by_b[b] = xt
    for c in range(8):
        b = c // 4
        hs = [4 * (c % 4) + j for j in range(NH_LOCAL)]
        q_rows = np.concatenate([np.arange(h * P, (h + 1) * P) for h in hs])
        k_rows = q_rows + H
        v_rows = q_rows + 2 * H

        def wtile(rows):
            # [p, ko, o] = w_qkv[rows[o], ko*128+p]
            w = w_qkv[rows, :]                      # [512, 2048]
            return np.ascontiguousarray(
                w.T.reshape(KO, P, len(rows)).transpose(1, 0, 2)
            ).astype(BF_NP)

        # wo[p, kb, o] = w_o[o, cols[kb*128+p]]
        wo_c = np.ascontiguousarray(
            w_o[:, q_rows].T.reshape(NH_LOCAL, P, S).transpose(1, 0, 2)
        ).astype(BF_NP)
        in_maps.append({
            "xt": xt_by_b[b],
            "wq": wtile(q_rows),
            "wk": wtile(k_rows),
            "wv": wtile(v_rows),
            "wo": wo_c,
        })
    return in_maps


def run(hidden_states, w_qkv, w_o, trace=False, trace_cores=None):
    in_maps = _prep_inputs(hidden_states, w_qkv, w_o)
    nc = _get_nc()
    kwargs = {}
    if trace:
        kwargs["trace_cores"] = (trace_cores if trace_cores is not None
                                 else list(range(8)))
    res = run_bass_kernel_spmd(nc, in_maps, core_ids=list(range(8)),
                               trace=trace, **kwargs)
    B, S_, H_ = np.asarray(hidden_states).shape
    out = np.zeros((B, S_, H_), dtype=np.float32)
    for c in range(8):
        b = c // 4
        outt = np.asarray(res.results[c]["outt"], dtype=np.float32)
        outT = outt.transpose(1, 0, 2).reshape(H_, S_)   # [128,16,2048]
        out[b] += outT.T
    return out, res


def kernel(hidden_states, w_qkv, w_o):
    out, _ = run(hidden_states, w_qkv, w_o, trace=False)
    return out


# revision 40
# speedup vs baseline: 1.0129x; 1.0129x over previous
"""Tensor-parallel causal attention kernel for 8 trn2 NeuronCores.

Problem: B=2, S=2048, H=2048, 16 heads, head_dim=128 fp32.
  qkv = hidden @ w_qkv.T ; causal attention ; out = attn @ w_o.T

Sharding (hardcoded): core c in 0..7 handles batch b=c//4 and heads
hs = [4*(c%4) .. 4*(c%4)+3].  Each core computes a partial o_proj
output (contraction over its 512 hidden dims); the host sums the 4
partials per batch and transposes.  No device collectives.

Device-side layout (all host-pretiled to partition-major [128, ...]):
  xt  [128,16,2048] f32 : xt[p,ko,s]  = hidden[b, s, ko*128+p]
  wq  [128,16, 512] f32 : wq[p,ko,o]  = w_qkv[q_rows[o],  ko*128+p]
  wk  [128,16, 512] f32 : wk[p,ko,o]  = w_qkv[k_rows[o],  ko*128+p]
  wv  [128,16, 512] f32 : wv[p,ko,d]  = w_qkv[v_rows[d],  ko*128+p]
  wo  [128, 4,2048] f32 : wo[p,kb,o]  = w_o[o, cols[kb*128+p]]
  outt[128,16,2048] f32 : outt[p,ot,s] = outT_partial[ot*128+p, s]

Design (single interleaved pipeline over four 512-token chunks):
  p1(0), p2(0), p1(1), p3(0), p2(1), p1(2), p3(1), ... p2(3), p3(3)
  - p1 projects one x chunk (Q,K columns + V blocks); psum->sbuf casts
    run on the scalar engine (idle during p1), prologue DMAs are
    per-ko-striped so the first V group trickles behind the DMA wave.
  - p2 is software-pipelined causal attention: sum+attnV consumers
    trail scores+exp by two k-pairs; diagonal blocks are processed
    first and stream only their valid q range (block-128-granular
    causality, bit-identical results); softmax sums use a [P,16] ones
    stationary so the PSUM drain spreads across 16 partitions.
  - p3 o_proj partials stage through bf16 sbuf tiles (10 bufs) and DMA
    out in bf16; the host accumulates the 4 partials per batch in f32.

Toolchain quirk workarounds (walrus 1-sync-wait slot):
  - chunked tail drain monkeypatch (single sem-only barrier, no
    trailing barrier)
  - post-pass hoisting extra sem waits onto same-engine NoOps
"""
import numpy as np

import concourse.bass as bass
import concourse.mybir as mybir
import concourse.tile as tile
from concourse.bass_utils import run_bass_kernel_spmd
from concourse.vector_clock import ScopedClock, VectorClock

P = 128
S = 2048
H = 2048
NH_LOCAL = 4          # heads per core
KO = H // P           # 16 contraction chunks for the projections
SQ = 512              # q chunk width
NQC = S // SQ         # 4 q chunks
NKB = S // P          # 16 key blocks
F32 = mybir.dt.float32
BF = mybir.dt.bfloat16
AF = mybir.ActivationFunctionType
SCALE = 1.0 / float(np.sqrt(128.0))

XCH = SQ              # x chunk width == q chunk width (interleaved pipe)
NXCH = S // XCH


def _drain_and_barrier_chunked(self, tick_clock, wait_clock, _MAX=1):
    """Split the kernel-tail drain's waits: walrus allows only one sync
    wait per instruction in this toolchain."""
    g = tick_clock.global_clock
    n = len(g)
    vals = [g[i] for i in range(n)]
    nz = [i for i, v in enumerate(vals) if v > 0]
    chunks = [nz[i:i + _MAX] for i in range(0, len(nz), _MAX)] or [[]]
    for chunk in chunks:
        vec = [vals[i] if i in chunk else 0 for i in range(n)]
        d = self.nc.sync.drain()
        wait_clock.add_sem_waits(d.ins, ScopedClock({None: VectorClock(vec)}))
    self.nc.all_engine_barrier(sem_only=True)
    assert self.sems is not None
    popped = self.nc._tile_sem_poison_stack.pop()
    assert popped is self._sem_poison
    # no trailing barrier: engines are already synced by the barrier
    # above, and the sem clear below is the true last operation.
    self.nc.clear_and_free_semaphores(list(self.sems.allocated().values()))


tile.TileContext._drain_and_barrier = _drain_and_barrier_chunked


def _split_multi_waits(nc):
    """walrus allows ONE sync wait per instruction: hoist extra waits onto
    same-engine NoOps inserted directly before the offending instruction
    (identical semantics — the engine queue blocks on each in turn)."""
    ctr = 0
    for f in nc.m.functions:
        for blk in f.blocks:
            new = []
            changed = False
            for inst in blk.instructions:
                si = inst.sync_info
                waits = list(si.on_wait) if si and si.on_wait else []
                if len(waits) > 1:
                    changed = True
                    for w in waits[:-1]:
                        ctr += 1
                        nop = mybir.InstNoOp(name=f"I-wsplit-{ctr}",
                                             engine=inst.engine,
                                             ins=[], outs=[])
                        nop.sync_info = mybir.SyncInfo(on_wait=[w],
                                                       on_update=[])
                        new.append(nop)
                    ups = list(si.on_update) if si.on_update else []
                    inst.sync_info = mybir.SyncInfo(on_wait=[waits[-1]],
                                                    on_update=ups)
                new.append(inst)
            if changed:
                blk.instructions = new
    return ctr


def build():
    nc = bass.Bass()
    xt = nc.dram_tensor("xt", [P, KO, S], BF, kind="ExternalInput")
    wq = nc.dram_tensor("wq", [P, KO, NH_LOCAL * P], BF, kind="ExternalInput")
    wk = nc.dram_tensor("wk", [P, KO, NH_LOCAL * P], BF, kind="ExternalInput")
    wv = nc.dram_tensor("wv", [P, KO, NH_LOCAL * P], BF, kind="ExternalInput")
    wo = nc.dram_tensor("wo", [P, NH_LOCAL, S], BF, kind="ExternalInput")
    outt = nc.dram_tensor("outt", [P, KO, S], BF, kind="ExternalOutput")

    with tile.TileContext(nc) as tc:
        from contextlib import ExitStack
        with ExitStack() as ctx:
            const = ctx.enter_context(tc.tile_pool(name="const", bufs=1))
            ones16_bf = const.tile([P, 16], BF)
            nc.vector.memset(ones16_bf[:], 1.0)
            onesrow_bf = const.tile([1, P], BF)
            nc.vector.memset(onesrow_bf[:], 1.0)

            # ---- resident SBUF tensors --------------------------------
            res = ctx.enter_context(tc.tile_pool(name="res", bufs=1))
            # Q,K as qkvT: [d_in, ot(0-3 Q heads, 4-7 K heads), s]
            qk_sb = res.tile([P, 2 * NH_LOCAL, S], BF)
            # V as [s_in, kb, d_local]
            v_sb = res.tile([P, NKB, NH_LOCAL * P], BF)
            attnT = res.tile([P, NH_LOCAL, S], BF)
            w_r = res.tile([P, KO, 3 * NH_LOCAL * P], BF)   # Q|K|V weights
            wo_r = res.tile([P, NH_LOCAL, S], BF)

            xp = ctx.enter_context(tc.tile_pool(name="xp", bufs=2))
            estp = ctx.enter_context(tc.tile_pool(name="estp", bufs=8))
            smallp = ctx.enter_context(tc.tile_pool(name="smallp", bufs=2))
            stagep = ctx.enter_context(tc.tile_pool(name="stagep", bufs=10))

            ps_big = ctx.enter_context(
                tc.tile_pool(name="ps_big", bufs=2, space="PSUM"))
            ps_sm = ctx.enter_context(
                tc.tile_pool(name="ps_sm", bufs=2, space="PSUM"))
            ps_sum = ctx.enter_context(
                tc.tile_pool(name="ps_sum", bufs=2, space="PSUM"))

            # ---- prologue DMAs ----------------------------------------
            # x0 + wv striped per-ko so the first V group's k-loop can
            # trickle behind the DMA wavefront (dma queues round-robin;
            # strip k lands before strip k+8).  wq follows in quarters so
            # the Q groups can start right after the V groups drain.
            x_tiles = {}
            x0 = xp.tile([P, KO, XCH], BF, tag="xr", name="xr0")
            # single-ko strips for the first 4 ko (earliest possible
            # first matmul), then 2-ko strips (2KB dma lines)
            strip_slices = ([slice(k, k + 1) for k in range(4)]
                            + [slice(k, k + 2) for k in range(4, KO, 2)])
            for ks in strip_slices:
                nc.sync.dma_start(x0[:, ks], xt.ap()[:, ks, 0:XCH])
                nc.sync.dma_start(
                    w_r[:, ks, 2 * NH_LOCAL * P:3 * NH_LOCAL * P],
                    wv.ap()[:, ks])
            x_tiles[0] = x0
            for kq in range(4):
                ks = slice(4 * kq, 4 * (kq + 1))
                nc.sync.dma_start(w_r[:, ks, 0:NH_LOCAL * P],
                                  wq.ap()[:, ks])
            for kq in range(4):
                ks = slice(4 * kq, 4 * (kq + 1))
                nc.sync.dma_start(
                    w_r[:, ks, NH_LOCAL * P:2 * NH_LOCAL * P],
                    wk.ap()[:, ks])
            nc.sync.dma_start(wo_r[:], wo.ap())

            # ---- phase helpers ----------------------------------------
            def p1_chunk(xc):
                """Project x chunk xc -> Q,K chunk cols + V blocks.
                PSUM->SBUF casts go to the scalar engine (idle here),
                keeping the DVE queue clear for p2/p3 work."""
                if xc + 1 < NXCH:     # prefetch next x chunk (4 queues)
                    x_n = xp.tile([P, KO, XCH], BF, tag="xr")
                    for kq in range(4):
                        ks = slice(4 * kq, 4 * (kq + 1))
                        nc.sync.dma_start(
                            x_n[:, ks],
                            xt.ap()[:, ks, (xc + 1) * XCH:(xc + 2) * XCH])
                    x_tiles[xc + 1] = x_n
                x_r = x_tiles[xc]
                # V first (st blocks): out [s=128, d=512]
                for st in range(XCH // P):
                    kb = xc * (XCH // P) + st
                    ps = ps_big.tile([P, 2 * SQ], F32, tag="big")
                    for k in range(KO):
                        nc.tensor.matmul(
                            ps[:, 0:NH_LOCAL * P],
                            x_r[:, k, st * P:(st + 1) * P],
                            w_r[:, k, 2 * NH_LOCAL * P:3 * NH_LOCAL * P],
                            start=(k == 0), stop=(k == KO - 1))
                    nc.scalar.copy(v_sb[:, kb, :], ps[:, 0:NH_LOCAL * P])
                # Q then K o-tiles: out [o=128, s=XCH]
                for ot in range(2 * NH_LOCAL):
                    ps = ps_big.tile([P, 2 * SQ], F32, tag="big")
                    for k in range(KO):
                        nc.tensor.matmul(
                            ps[:, 0:XCH], w_r[:, k, ot * P:(ot + 1) * P],
                            x_r[:, k], start=(k == 0), stop=(k == KO - 1))
                    nc.scalar.copy(
                        qk_sb[:, ot, xc * XCH:(xc + 1) * XCH], ps[:, 0:XCH])

            def p2_chunk(qc):
                """Causal attention for q chunk qc, all local heads.
                Software-pipelined: consumers (sum+attnV) of pair g are
                emitted two pairs behind scores+exp.  Diagonal blocks are
                processed FIRST so the end-of-head flush lands on pairs
                whose est needs no gpsimd mask; they also stream only
                their valid q range [off, SQ) — the skipped columns are
                exactly the all-masked (zero) ones.  The first processed
                block (kb=4*qc) has off==0, so the start=True matmuls
                initialize every column."""
                nkb = 4 * (qc + 1)
                npair = nkb // 2
                qs = qc * SQ
                # processed block order: 4 diagonal blocks, then the rest
                blocks = list(range(nkb - 4, nkb)) + list(range(0, nkb - 4))
                state = {}            # h -> (at_ps, sm_ps)
                pending = []          # [(h, pi, est)], consumers lag 2

                def consumers(h, pi, est):
                    at_ps, sm_ps = state[h]
                    # both sums back-to-back: one ones-weights load
                    for j in range(2):
                        pos = 2 * pi + j
                        kb = blocks[pos]
                        off = max(0, kb * P - qs)
                        nc.tensor.matmul(sm_ps[0:16, off:SQ], ones16_bf[:],
                                         est[:, j * SQ + off:(j + 1) * SQ],
                                         start=(pos == 0),
                                         stop=(pos == nkb - 1))
                    for j in range(2):
                        pos = 2 * pi + j
                        kb = blocks[pos]
                        off = max(0, kb * P - qs)
                        nc.tensor.matmul(
                            at_ps[:, off:SQ],
                            v_sb[:, kb, h * P:(h + 1) * P],
                            est[:, j * SQ + off:(j + 1) * SQ],
                            start=(pos == 0), stop=(pos == nkb - 1))
                    if pi == npair - 1:   # head tail
                        logs = smallp.tile([1, SQ], F32, tag="logs")
                        nc.scalar.activation(logs[:], sm_ps[0:1, :], AF.Ln)
                        recip = smallp.tile([1, SQ], BF, tag="recip")
                        nc.scalar.activation(recip[:], logs[:], AF.Exp,
                                             scale=-1.0)
                        # broadcast recip to 128 partitions via K=1 matmul
                        # (reuses the sum-tag psum slot; Ln has already
                        # read the old contents by the time this runs)
                        rep_ps = ps_sum.tile([P, SQ], F32, tag="sum",
                                             name=f"rep{qc}_{h}")
                        nc.tensor.matmul(rep_ps[:], onesrow_bf[:], recip[:],
                                         start=True, stop=True)
                        rep_sb = smallp.tile([P, SQ], F32, tag="rep")
                        nc.vector.tensor_copy(rep_sb[:], rep_ps[:])
                        nc.vector.tensor_mul(attnT[:, h, qs:qs + SQ],
                                             at_ps[:], rep_sb[:])
                        del state[h]

                for h in range(NH_LOCAL):
                    state[h] = (ps_sm.tile([P, SQ], F32, tag="small", name=f"at{qc}_{h}"),
                                ps_sum.tile([P, SQ], F32, tag="sum", name=f"sm{qc}_{h}"))
                    for pi in range(npair):
                        b0, b1 = blocks[2 * pi], blocks[2 * pi + 1]
                        st_ps = ps_big.tile([P, 2 * SQ], F32, tag="big")
                        for j, kb in enumerate((b0, b1)):
                            off = max(0, kb * P - qs)
                            nc.tensor.matmul(
                                st_ps[:, j * SQ + off:(j + 1) * SQ],
                                qk_sb[:, NH_LOCAL + h, kb * P:(kb + 1) * P],
                                qk_sb[:, h, qs + off:qs + SQ],
                                start=True, stop=True)
                        est = estp.tile([P, 2 * SQ], BF, tag="est")
                        off0 = max(0, b0 * P - qs)
                        # [SQ, SQ+off1) holds never-written garbage; its
                        # exp output is never read (consumers start at
                        # off1 within block j=1).
                        nc.scalar.activation(est[:, off0:2 * SQ],
                                             st_ps[:, off0:2 * SQ], AF.Exp,
                                             scale=SCALE)
                        for j, kb in enumerate((b0, b1)):
                            off = max(0, kb * P - qs)
                            if kb * P + P - 1 > qs:  # crosses diagonal
                                nc.gpsimd.affine_select(
                                    est[:, j * SQ + off:(j + 1) * SQ],
                                    est[:, j * SQ + off:(j + 1) * SQ],
                                    [[1, SQ - off]],
                                    mybir.AluOpType.is_ge, 0.0,
                                    base=qs + off - kb * P,
                                    channel_multiplier=-1)
                        if len(pending) >= 3:
                            consumers(*pending.pop(0))
                        pending.append((h, pi, est))
                while pending:
                    consumers(*pending.pop(0))

            def p3_chunk(sc):
                """o_proj partial for q chunk sc."""
                for ot in range(KO):
                    ps = ps_sm.tile([P, SQ], F32, tag="small")
                    for kb in range(NH_LOCAL):
                        nc.tensor.matmul(
                            ps[:], wo_r[:, kb, ot * P:(ot + 1) * P],
                            attnT[:, kb, sc * SQ:(sc + 1) * SQ],
                            start=(kb == 0), stop=(kb == NH_LOCAL - 1))
                    stage = stagep.tile([P, SQ], BF, tag="p3stage")
                    nc.vector.tensor_copy(stage[:], ps[:])
                    nc.sync.dma_start(
                        outt.ap()[:, ot, sc * SQ:(sc + 1) * SQ], stage[:])

            # ---- interleaved pipeline ---------------------------------
            # p1(0), p2(0), p1(1), p3(0), p2(1), p1(2), p3(1), ...
            p1_chunk(0)
            for xc in range(NXCH):
                p2_chunk(xc)
                if xc + 1 < NXCH:
                    p1_chunk(xc + 1)
                p3_chunk(xc)
    _split_multi_waits(nc)
    return nc


_NC_CACHE = None


def _get_nc():
    global _NC_CACHE
    if _NC_CACHE is None:
        _NC_CACHE = build()
    return _NC_CACHE


def _prep_inputs(hidden_states, w_qkv, w_o):
    """Host-side shard + pre-tile + bf16-cast for the 8 cores."""
    import ml_dtypes
    BF_NP = ml_dtypes.bfloat16
    hidden_states = np.asarray(hidden_states, dtype=np.float32)
    w_qkv = np.asarray(w_qkv, dtype=np.float32)
    w_o = np.asarray(w_o, dtype=np.float32)
    B = hidden_states.shape[0]

    in_maps = []
    xt_by_b = {}
    for b in range(B):
        # xt[p, ko, s] = hidden[b, s, ko*128+p]
        xt = np.ascontiguousarray(
            hidden_states[b].T.reshape(KO, P, S).transpose(1, 0, 2)
        ).astype(BF_NP)
        xt_by_b[b] = xt
    for c in range(8):
        b = c // 4
        hs = [4 * (c % 4) + j for j in range(NH_LOCAL)]
        q_rows = np.concatenate([np.arange(h * P, (h + 1) * P) for h in hs])
        k_rows = q_rows + H
        v_rows = q_rows + 2 * H

        def wtile(rows):
            # [p, ko, o] = w_qkv[rows[o], ko*128+p]
            w = w_qkv[rows, :]                      # [512, 2048]
            return np.ascontiguousarray(
                w.T.reshape(KO, P, len(rows)).transpose(1, 0, 2)
            ).astype(BF_NP)

        # wo[p, kb, o] = w_o[o, cols[kb*128+p]]
        wo_c = np.ascontiguousarray(
            w_o[:, q_rows].T.reshape(NH_LOCAL, P, S).transpose(1, 0, 2)
        ).astype(BF_NP)
        in_maps.append({
            "xt": xt_by_b[b],
            "wq": wtile(q_rows),
            "wk": wtile(k_rows),
            "wv": wtile(v_rows),
            "wo": wo_c,
        })
    return in_maps


def run(hidden_states, w_qkv, w_o, trace=False, trace_cores=None):
    in_maps = _prep_inputs(hidden_states, w_qkv, w_o)
    nc = _get_nc()
    kwargs = {}
    if trace:
        kwargs["trace_cores"] = (trace_cores if trace_cores is not None
                                 else list(range(8)))
    res = run_bass_kernel_spmd(nc, in_maps, core_ids=list(range(8)),
                               trace=trace, **kwargs)
    B, S_, H_ = np.asarray(hidden_states).shape
    out = np.zeros((B, S_, H_), dtype=np.float32)
    for c in range(8):
        b = c // 4
        outt = np.asarray(res.results[c]["outt"], dtype=np.float32)
        outT = outt.transpose(1, 0, 2).reshape(H_, S_)   # [128,16,2048]
        out[b] += outT.T
    return out, res


def kernel(hidden_states, w_qkv, w_o):
    out, _ = run(hidden_states, w_qkv, w_o, trace=False)
    return out


# revision 41
# speedup vs baseline: 1.0140x; 1.0010x over previous
"""Tensor-parallel causal attention kernel for 8 trn2 NeuronCores.

Problem: B=2, S=2048, H=2048, 16 heads, head_dim=128 fp32.
  qkv = hidden @ w_qkv.T ; causal attention ; out = attn @ w_o.T

Sharding (hardcoded): core c in 0..7 handles batch b=c//4 and heads
hs = [4*(c%4) .. 4*(c%4)+3].  Each core computes a partial o_proj
output (contraction over its 512 hidden dims); the host sums the 4
partials per batch and transposes.  No device collectives.

Device-side layout (all host-pretiled to partition-major [128, ...]):
  xt  [128,16,2048] f32 : xt[p,ko,s]  = hidden[b, s, ko*128+p]
  wq  [128,16, 512] f32 : wq[p,ko,o]  = w_qkv[q_rows[o],  ko*128+p]
  wk  [128,16, 512] f32 : wk[p,ko,o]  = w_qkv[k_rows[o],  ko*128+p]
  wv  [128,16, 512] f32 : wv[p,ko,d]  = w_qkv[v_rows[d],  ko*128+p]
  wo  [128, 4,2048] f32 : wo[p,kb,o]  = w_o[o, cols[kb*128+p]]
  outt[128,16,2048] f32 : outt[p,ot,s] = outT_partial[ot*128+p, s]

Design (single interleaved pipeline over four 512-token chunks):
  p1(0), p2(0), p1(1), p3(0), p2(1), p1(2), p3(1), ... p2(3), p3(3)
  - p1 projects one x chunk (Q,K columns + V blocks); psum->sbuf casts
    run on the scalar engine (idle during p1), prologue DMAs are
    per-ko-striped so the first V group trickles behind the DMA wave.
  - p2 is software-pipelined causal attention: sum+attnV consumers
    trail scores+exp by two k-pairs; diagonal blocks are processed
    first and stream only their valid q range (block-128-granular
    causality, bit-identical results); softmax sums use a [P,16] ones
    stationary so the PSUM drain spreads across 16 partitions.
  - p3 o_proj partials stage through bf16 sbuf tiles (10 bufs) and DMA
    out in bf16; the host accumulates the 4 partials per batch in f32.

Toolchain quirk workarounds (walrus 1-sync-wait slot):
  - chunked tail drain monkeypatch (single sem-only barrier, no
    trailing barrier)
  - post-pass hoisting extra sem waits onto same-engine NoOps
"""
import numpy as np

import concourse.bass as bass
import concourse.mybir as mybir
import concourse.tile as tile
from concourse.bass_utils import run_bass_kernel_spmd
from concourse.vector_clock import ScopedClock, VectorClock

P = 128
S = 2048
H = 2048
NH_LOCAL = 4          # heads per core
KO = H // P           # 16 contraction chunks for the projections
SQ = 512              # q chunk width
NQC = S // SQ         # 4 q chunks
NKB = S // P          # 16 key blocks
F32 = mybir.dt.float32
BF = mybir.dt.bfloat16
AF = mybir.ActivationFunctionType
SCALE = 1.0 / float(np.sqrt(128.0))

XCH = SQ              # x chunk width == q chunk width (interleaved pipe)
NXCH = S // XCH


def _drain_and_barrier_chunked(self, tick_clock, wait_clock, _MAX=1):
    """Split the kernel-tail drain's waits: walrus allows only one sync
    wait per instruction in this toolchain."""
    g = tick_clock.global_clock
    n = len(g)
    vals = [g[i] for i in range(n)]
    nz = [i for i, v in enumerate(vals) if v > 0]
    chunks = [nz[i:i + _MAX] for i in range(0, len(nz), _MAX)] or [[]]
    for chunk in chunks:
        vec = [vals[i] if i in chunk else 0 for i in range(n)]
        d = self.nc.sync.drain()
        wait_clock.add_sem_waits(d.ins, ScopedClock({None: VectorClock(vec)}))
    self.nc.all_engine_barrier(sem_only=True)
    assert self.sems is not None
    popped = self.nc._tile_sem_poison_stack.pop()
    assert popped is self._sem_poison
    # no trailing barrier: engines are already synced by the barrier
    # above, and the sem clear below is the true last operation.
    self.nc.clear_and_free_semaphores(list(self.sems.allocated().values()))


tile.TileContext._drain_and_barrier = _drain_and_barrier_chunked


def _split_multi_waits(nc):
    """walrus allows ONE sync wait per instruction: hoist extra waits onto
    same-engine NoOps inserted directly before the offending instruction
    (identical semantics — the engine queue blocks on each in turn)."""
    ctr = 0
    for f in nc.m.functions:
        for blk in f.blocks:
            new = []
            changed = False
            for inst in blk.instructions:
                si = inst.sync_info
                waits = list(si.on_wait) if si and si.on_wait else []
                if len(waits) > 1:
                    changed = True
                    for w in waits[:-1]:
                        ctr += 1
                        nop = mybir.InstNoOp(name=f"I-wsplit-{ctr}",
                                             engine=inst.engine,
                                             ins=[], outs=[])
                        nop.sync_info = mybir.SyncInfo(on_wait=[w],
                                                       on_update=[])
                        new.append(nop)
                    ups = list(si.on_update) if si.on_update else []
                    inst.sync_info = mybir.SyncInfo(on_wait=[waits[-1]],
                                                    on_update=ups)
                new.append(inst)
            if changed:
                blk.instructions = new
    return ctr


def build():
    nc = bass.Bass()
    xt = nc.dram_tensor("xt", [P, KO, S], BF, kind="ExternalInput")
    wq = nc.dram_tensor("wq", [P, KO, NH_LOCAL * P], BF, kind="ExternalInput")
    wk = nc.dram_tensor("wk", [P, KO, NH_LOCAL * P], BF, kind="ExternalInput")
    wv = nc.dram_tensor("wv", [P, KO, NH_LOCAL * P], BF, kind="ExternalInput")
    wo = nc.dram_tensor("wo", [P, NH_LOCAL, S], BF, kind="ExternalInput")
    outt = nc.dram_tensor("outt", [P, KO, S], BF, kind="ExternalOutput")

    with tile.TileContext(nc) as tc:
        from contextlib import ExitStack
        with ExitStack() as ctx:
            const = ctx.enter_context(tc.tile_pool(name="const", bufs=1))
            ones16_bf = const.tile([P, 16], BF)
            nc.vector.memset(ones16_bf[:], 1.0)
            onesrow_bf = const.tile([1, P], BF)
            nc.vector.memset(onesrow_bf[:], 1.0)

            # ---- resident SBUF tensors --------------------------------
            res = ctx.enter_context(tc.tile_pool(name="res", bufs=1))
            # Q,K as qkvT: [d_in, ot(0-3 Q heads, 4-7 K heads), s]
            qk_sb = res.tile([P, 2 * NH_LOCAL, S], BF)
            # V as [s_in, kb, d_local]
            v_sb = res.tile([P, NKB, NH_LOCAL * P], BF)
            attnT = res.tile([P, NH_LOCAL, S], BF)
            w_r = res.tile([P, KO, 3 * NH_LOCAL * P], BF)   # Q|K|V weights
            wo_r = res.tile([P, NH_LOCAL, S], BF)

            xp = ctx.enter_context(tc.tile_pool(name="xp", bufs=2))
            estp = ctx.enter_context(tc.tile_pool(name="estp", bufs=8))
            smallp = ctx.enter_context(tc.tile_pool(name="smallp", bufs=2))
            stagep = ctx.enter_context(tc.tile_pool(name="stagep", bufs=10))

            ps_big = ctx.enter_context(
                tc.tile_pool(name="ps_big", bufs=2, space="PSUM"))
            ps_sm = ctx.enter_context(
                tc.tile_pool(name="ps_sm", bufs=2, space="PSUM"))
            ps_sum = ctx.enter_context(
                tc.tile_pool(name="ps_sum", bufs=2, space="PSUM"))

            # ---- prologue DMAs ----------------------------------------
            # x0 + wv striped per-ko so the first V group's k-loop can
            # trickle behind the DMA wavefront (dma queues round-robin;
            # strip k lands before strip k+8).  wq follows in quarters so
            # the Q groups can start right after the V groups drain.
            x_tiles = {}
            x0 = xp.tile([P, KO, XCH], BF, tag="xr", name="xr0")
            # single-ko strips for the first 4 ko (earliest possible
            # first matmul), then 2-ko strips (2KB dma lines)
            strip_slices = ([slice(k, k + 1) for k in range(4)]
                            + [slice(k, k + 2) for k in range(4, KO, 2)])
            for ks in strip_slices:
                nc.sync.dma_start(x0[:, ks], xt.ap()[:, ks, 0:XCH])
                nc.sync.dma_start(
                    w_r[:, ks, 2 * NH_LOCAL * P:3 * NH_LOCAL * P],
                    wv.ap()[:, ks])
            x_tiles[0] = x0
            for k2 in range(KO // 2):
                ks = slice(2 * k2, 2 * k2 + 2)
                nc.sync.dma_start(w_r[:, ks, 0:NH_LOCAL * P],
                                  wq.ap()[:, ks])
            for k2 in range(KO // 2):
                ks = slice(2 * k2, 2 * k2 + 2)
                nc.sync.dma_start(
                    w_r[:, ks, NH_LOCAL * P:2 * NH_LOCAL * P],
                    wk.ap()[:, ks])
            nc.sync.dma_start(wo_r[:], wo.ap())

            # ---- phase helpers ----------------------------------------
            def p1_chunk(xc):
                """Project x chunk xc -> Q,K chunk cols + V blocks.
                PSUM->SBUF casts go to the scalar engine (idle here),
                keeping the DVE queue clear for p2/p3 work."""
                if xc + 1 < NXCH:     # prefetch next x chunk (4 queues)
                    x_n = xp.tile([P, KO, XCH], BF, tag="xr")
                    for kq in range(4):
                        ks = slice(4 * kq, 4 * (kq + 1))
                        nc.sync.dma_start(
                            x_n[:, ks],
                            xt.ap()[:, ks, (xc + 1) * XCH:(xc + 2) * XCH])
                    x_tiles[xc + 1] = x_n
                x_r = x_tiles[xc]
                # V first (st blocks): out [s=128, d=512]
                for st in range(XCH // P):
                    kb = xc * (XCH // P) + st
                    ps = ps_big.tile([P, 2 * SQ], F32, tag="big")
                    for k in range(KO):
                        nc.tensor.matmul(
                            ps[:, 0:NH_LOCAL * P],
                            x_r[:, k, st * P:(st + 1) * P],
                            w_r[:, k, 2 * NH_LOCAL * P:3 * NH_LOCAL * P],
                            start=(k == 0), stop=(k == KO - 1))
                    nc.scalar.copy(v_sb[:, kb, :], ps[:, 0:NH_LOCAL * P])
                # Q then K o-tiles: out [o=128, s=XCH]
                for ot in range(2 * NH_LOCAL):
                    ps = ps_big.tile([P, 2 * SQ], F32, tag="big")
                    for k in range(KO):
                        nc.tensor.matmul(
                            ps[:, 0:XCH], w_r[:, k, ot * P:(ot + 1) * P],
                            x_r[:, k], start=(k == 0), stop=(k == KO - 1))
                    nc.scalar.copy(
                        qk_sb[:, ot, xc * XCH:(xc + 1) * XCH], ps[:, 0:XCH])

            def p2_chunk(qc):
                """Causal attention for q chunk qc, all local heads.
                Software-pipelined: consumers (sum+attnV) of pair g are
                emitted two pairs behind scores+exp.  Diagonal blocks are
                processed FIRST so the end-of-head flush lands on pairs
                whose est needs no gpsimd mask; they also stream only
                their valid q range [off, SQ) — the skipped columns are
                exactly the all-masked (zero) ones.  The first processed
                block (kb=4*qc) has off==0, so the start=True matmuls
                initialize every column."""
                nkb = 4 * (qc + 1)
                npair = nkb // 2
                qs = qc * SQ
                # processed block order: 4 diagonal blocks, then the rest
                blocks = list(range(nkb - 4, nkb)) + list(range(0, nkb - 4))
                state = {}            # h -> (at_ps, sm_ps)
                pending = []          # [(h, pi, est)], consumers lag 2

                def consumers(h, pi, est):
                    at_ps, sm_ps = state[h]
                    # both sums back-to-back: one ones-weights load
                    for j in range(2):
                        pos = 2 * pi + j
                        kb = blocks[pos]
                        off = max(0, kb * P - qs)
                        nc.tensor.matmul(sm_ps[0:16, off:SQ], ones16_bf[:],
                                         est[:, j * SQ + off:(j + 1) * SQ],
                                         start=(pos == 0),
                                         stop=(pos == nkb - 1))
                    for j in range(2):
                        pos = 2 * pi + j
                        kb = blocks[pos]
                        off = max(0, kb * P - qs)
                        nc.tensor.matmul(
                            at_ps[:, off:SQ],
                            v_sb[:, kb, h * P:(h + 1) * P],
                            est[:, j * SQ + off:(j + 1) * SQ],
                            start=(pos == 0), stop=(pos == nkb - 1))
                    if pi == npair - 1:   # head tail
                        logs = smallp.tile([1, SQ], F32, tag="logs")
                        nc.scalar.activation(logs[:], sm_ps[0:1, :], AF.Ln)
                        recip = smallp.tile([1, SQ], BF, tag="recip")
                        nc.scalar.activation(recip[:], logs[:], AF.Exp,
                                             scale=-1.0)
                        # broadcast recip to 128 partitions via K=1 matmul
                        # (reuses the sum-tag psum slot; Ln has already
                        # read the old contents by the time this runs)
                        rep_ps = ps_sum.tile([P, SQ], F32, tag="sum",
                                             name=f"rep{qc}_{h}")
                        nc.tensor.matmul(rep_ps[:], onesrow_bf[:], recip[:],
                                         start=True, stop=True)
                        rep_sb = smallp.tile([P, SQ], F32, tag="rep")
                        nc.vector.tensor_copy(rep_sb[:], rep_ps[:])
                        nc.vector.tensor_mul(attnT[:, h, qs:qs + SQ],
                                             at_ps[:], rep_sb[:])
                        del state[h]

                for h in range(NH_LOCAL):
                    state[h] = (ps_sm.tile([P, SQ], F32, tag="small", name=f"at{qc}_{h}"),
                                ps_sum.tile([P, SQ], F32, tag="sum", name=f"sm{qc}_{h}"))
                    for pi in range(npair):
                        b0, b1 = blocks[2 * pi], blocks[2 * pi + 1]
                        st_ps = ps_big.tile([P, 2 * SQ], F32, tag="big")
                        for j, kb in enumerate((b0, b1)):
                            off = max(0, kb * P - qs)
                            nc.tensor.matmul(
                                st_ps[:, j * SQ + off:(j + 1) * SQ],
                                qk_sb[:, NH_LOCAL + h, kb * P:(kb + 1) * P],
                                qk_sb[:, h, qs + off:qs + SQ],
                                start=True, stop=True)
                        est = estp.tile([P, 2 * SQ], BF, tag="est")
                        off0 = max(0, b0 * P - qs)
                        # [SQ, SQ+off1) holds never-written garbage; its
                        # exp output is never read (consumers start at
                        # off1 within block j=1).
                        nc.scalar.activation(est[:, off0:2 * SQ],
                                             st_ps[:, off0:2 * SQ], AF.Exp,
                                             scale=SCALE)
                        for j, kb in enumerate((b0, b1)):
                            off = max(0, kb * P - qs)
                            if kb * P + P - 1 > qs:  # crosses diagonal
                                nc.gpsimd.affine_select(
                                    est[:, j * SQ + off:(j + 1) * SQ],
                                    est[:, j * SQ + off:(j + 1) * SQ],
                                    [[1, SQ - off]],
                                    mybir.AluOpType.is_ge, 0.0,
                                    base=qs + off - kb * P,
                                    channel_multiplier=-1)
                        if len(pending) >= 3:
                            consumers(*pending.pop(0))
                        pending.append((h, pi, est))
                while pending:
                    consumers(*pending.pop(0))

            def p3_chunk(sc):
                """o_proj partial for q chunk sc."""
                for ot in range(KO):
                    ps = ps_sm.tile([P, SQ], F32, tag="small")
                    for kb in range(NH_LOCAL):
                        nc.tensor.matmul(
                            ps[:], wo_r[:, kb, ot * P:(ot + 1) * P],
                            attnT[:, kb, sc * SQ:(sc + 1) * SQ],
                            start=(kb == 0), stop=(kb == NH_LOCAL - 1))
                    stage = stagep.tile([P, SQ], BF, tag="p3stage")
                    nc.vector.tensor_copy(stage[:], ps[:])
                    nc.sync.dma_start(
                        outt.ap()[:, ot, sc * SQ:(sc + 1) * SQ], stage[:])

            # ---- interleaved pipeline ---------------------------------
            # p1(0), p2(0), p1(1), p3(0), p2(1), p1(2), p3(1), ...
            p1_chunk(0)
            for xc in range(NXCH):
                p2_chunk(xc)
                if xc + 1 < NXCH:
                    p1_chunk(xc + 1)
                p3_chunk(xc)
    _split_multi_waits(nc)
    return nc


_NC_CACHE = None


def _get_nc():
    global _NC_CACHE
    if _NC_CACHE is None:
        _NC_CACHE = build()
    return _NC_CACHE


def _prep_inputs(hidden_states, w_qkv, w_o):
    """Host-side shard + pre-tile + bf16-cast for the 8 cores."""
    import ml_dtypes
    BF_NP = ml_dtypes.bfloat16
    hidden_states = np.asarray(hidden_states, dtype=np.float32)
    w_qkv = np.asarray(w_qkv, dtype=np.float32)
    w_o = np.asarray(w_o, dtype=np.float32)
    B = hidden_states.shape[0]

    in_maps = []
    xt_by_b = {}
    for b in range(B):
        # xt[p, ko, s] = hidden[b, s, ko*128+p]
        xt = np.ascontiguousarray(
            hidden_states[b].T.reshape(KO, P, S).transpose(1, 0, 2)
        ).astype(BF_NP)
        xt_by_b[b] = xt
    for c in range(8):
        b = c // 4
        hs = [4 * (c % 4) + j for j in range(NH_LOCAL)]
        q_rows = np.concatenate([np.arange(h * P, (h + 1) * P) for h in hs])
        k_rows = q_rows + H
        v_rows = q_rows + 2 * H

        def wtile(rows):
            # [p, ko, o] = w_qkv[rows[o], ko*128+p]
            w = w_qkv[rows, :]                      # [512, 2048]
            return np.ascontiguousarray(
                w.T.reshape(KO, P, len(rows)).transpose(1, 0, 2)
            ).astype(BF_NP)

        # wo[p, kb, o] = w_o[o, cols[kb*128+p]]
        wo_c = np.ascontiguousarray(
            w_o[:, q_rows].T.reshape(NH_LOCAL, P, S).transpose(1, 0, 2)
        ).astype(BF_NP)
        in_maps.append({
            "xt": xt_by_b[b],
            "wq": wtile(q_rows),
            "wk": wtile(k_rows),
            "wv": wtile(v_rows),
            "wo": wo_c,
        })
    return in_maps


def run(hidden_states, w_qkv, w_o, trace=False, trace_cores=None):
    in_maps = _prep_inputs(hidden_states, w_qkv, w_o)
    nc = _get_nc()
    kwargs = {}
    if trace:
        kwargs["trace_cores"] = (trace_cores if trace_cores is not None
                                 else list(range(8)))
    res = run_bass_kernel_spmd(nc, in_maps, core_ids=list(range(8)),
                               trace=trace, **kwargs)
    B, S_, H_ = np.asarray(hidden_states).shape
    out = np.zeros((B, S_, H_), dtype=np.float32)
    for c in range(8):
        b = c // 4
        outt = np.asarray(res.results[c]["outt"], dtype=np.float32)
        outT = outt.transpose(1, 0, 2).reshape(H_, S_)   # [128,16,2048]
        out[b] += outT.T
    return out, res


def kernel(hidden_states, w_qkv, w_o):
    out, _ = run(hidden_states, w_qkv, w_o, trace=False)
    return out


# revision 43
# speedup vs baseline: 1.0225x; 1.0084x over previous
"""Tensor-parallel causal attention kernel for 8 trn2 NeuronCores.

Problem: B=2, S=2048, H=2048, 16 heads, head_dim=128 fp32.
  qkv = hidden @ w_qkv.T ; causal attention ; out = attn @ w_o.T

Sharding (hardcoded): core c in 0..7 handles batch b=c//4 and heads
hs = [4*(c%4) .. 4*(c%4)+3].  Each core computes a partial o_proj
output (contraction over its 512 hidden dims); the host sums the 4
partials per batch and transposes.  No device collectives.

Device-side layout (all host-pretiled to partition-major [128, ...]):
  xt  [128,16,2048] f32 : xt[p,ko,s]  = hidden[b, s, ko*128+p]
  wq  [128,16, 512] f32 : wq[p,ko,o]  = w_qkv[q_rows[o],  ko*128+p]
  wk  [128,16, 512] f32 : wk[p,ko,o]  = w_qkv[k_rows[o],  ko*128+p]
  wv  [128,16, 512] f32 : wv[p,ko,d]  = w_qkv[v_rows[d],  ko*128+p]
  wo  [128, 4,2048] f32 : wo[p,kb,o]  = w_o[o, cols[kb*128+p]]
  outt[128,16,2048] f32 : outt[p,ot,s] = outT_partial[ot*128+p, s]

Design (single interleaved pipeline over four 512-token chunks):
  p1(0), p2(0), p1(1), p3(0), p2(1), p1(2), p3(1), ... p2(3), p3(3)
  - p1 projects one x chunk (Q,K columns + V blocks); psum->sbuf casts
    run on the scalar engine (idle during p1), prologue DMAs are
    per-ko-striped so the first V group trickles behind the DMA wave.
  - p2 is software-pipelined causal attention: sum+attnV consumers
    trail scores+exp by two k-pairs; diagonal blocks are processed
    first and stream only their valid q range (block-128-granular
    causality, bit-identical results); softmax sums use a [P,16] ones
    stationary so the PSUM drain spreads across 16 partitions.
  - p3 o_proj partials stage through bf16 sbuf tiles (10 bufs) and DMA
    out in bf16; the host accumulates the 4 partials per batch in f32.

Toolchain quirk workarounds (walrus 1-sync-wait slot):
  - chunked tail drain monkeypatch (single sem-only barrier, no
    trailing barrier)
  - post-pass hoisting extra sem waits onto same-engine NoOps
"""
import numpy as np

import concourse.bass as bass
import concourse.mybir as mybir
import concourse.tile as tile
from concourse.bass_utils import run_bass_kernel_spmd
from concourse.vector_clock import ScopedClock, VectorClock

P = 128
S = 2048
H = 2048
NH_LOCAL = 4          # heads per core
KO = H // P           # 16 contraction chunks for the projections
SQ = 512              # q chunk width
NQC = S // SQ         # 4 q chunks
NKB = S // P          # 16 key blocks
F32 = mybir.dt.float32
BF = mybir.dt.bfloat16
AF = mybir.ActivationFunctionType
SCALE = 1.0 / float(np.sqrt(128.0))

XCH = SQ              # x chunk width == q chunk width (interleaved pipe)
NXCH = S // XCH


def _drain_and_barrier_chunked(self, tick_clock, wait_clock, _MAX=1):
    """Split the kernel-tail drain's waits: walrus allows only one sync
    wait per instruction in this toolchain."""
    g = tick_clock.global_clock
    n = len(g)
    vals = [g[i] for i in range(n)]
    nz = [i for i, v in enumerate(vals) if v > 0]
    chunks = [nz[i:i + _MAX] for i in range(0, len(nz), _MAX)] or [[]]
    for chunk in chunks:
        vec = [vals[i] if i in chunk else 0 for i in range(n)]
        d = self.nc.sync.drain()
        wait_clock.add_sem_waits(d.ins, ScopedClock({None: VectorClock(vec)}))
    self.nc.all_engine_barrier(sem_only=True)
    assert self.sems is not None
    popped = self.nc._tile_sem_poison_stack.pop()
    assert popped is self._sem_poison
    # no trailing barrier: engines are already synced by the barrier
    # above, and the sem clear below is the true last operation.
    self.nc.clear_and_free_semaphores(list(self.sems.allocated().values()))


tile.TileContext._drain_and_barrier = _drain_and_barrier_chunked


def _split_multi_waits(nc):
    """walrus allows ONE sync wait per instruction: hoist extra waits onto
    same-engine NoOps inserted directly before the offending instruction
    (identical semantics — the engine queue blocks on each in turn)."""
    ctr = 0
    for f in nc.m.functions:
        for blk in f.blocks:
            new = []
            changed = False
            for inst in blk.instructions:
                si = inst.sync_info
                waits = list(si.on_wait) if si and si.on_wait else []
                if len(waits) > 1:
                    changed = True
                    for w in waits[:-1]:
                        ctr += 1
                        nop = mybir.InstNoOp(name=f"I-wsplit-{ctr}",
                                             engine=inst.engine,
                                             ins=[], outs=[])
                        nop.sync_info = mybir.SyncInfo(on_wait=[w],
                                                       on_update=[])
                        new.append(nop)
                    ups = list(si.on_update) if si.on_update else []
                    inst.sync_info = mybir.SyncInfo(on_wait=[waits[-1]],
                                                    on_update=ups)
                new.append(inst)
            if changed:
                blk.instructions = new
    return ctr


def build():
    nc = bass.Bass()
    xt = nc.dram_tensor("xt", [P, KO, S], BF, kind="ExternalInput")
    wq = nc.dram_tensor("wq", [P, KO, NH_LOCAL * P], BF, kind="ExternalInput")
    wk = nc.dram_tensor("wk", [P, KO, NH_LOCAL * P], BF, kind="ExternalInput")
    wv = nc.dram_tensor("wv", [P, KO, NH_LOCAL * P], BF, kind="ExternalInput")
    wo = nc.dram_tensor("wo", [P, NH_LOCAL, S], BF, kind="ExternalInput")
    outt = nc.dram_tensor("outt", [P, KO, S], BF, kind="ExternalOutput")

    with tile.TileContext(nc) as tc:
        from contextlib import ExitStack
        with ExitStack() as ctx:
            const = ctx.enter_context(tc.tile_pool(name="const", bufs=1))
            ones16_bf = const.tile([P, 16], BF)
            nc.vector.memset(ones16_bf[:], 1.0)
            onesrow_bf = const.tile([1, P], BF)
            nc.vector.memset(onesrow_bf[:], 1.0)

            # ---- resident SBUF tensors --------------------------------
            res = ctx.enter_context(tc.tile_pool(name="res", bufs=1))
            # Q,K as qkvT: [d_in, ot(0-3 Q heads, 4-7 K heads), s]
            qk_sb = res.tile([P, 2 * NH_LOCAL, S], BF)
            # V as [s_in, kb, d_local]
            v_sb = res.tile([P, NKB, NH_LOCAL * P], BF)
            attnT = res.tile([P, NH_LOCAL, S], BF)
            w_r = res.tile([P, KO, 3 * NH_LOCAL * P], BF)   # Q|K|V weights
            wo_r = res.tile([P, NH_LOCAL, S], BF)

            xp = ctx.enter_context(tc.tile_pool(name="xp", bufs=2))
            estp = ctx.enter_context(tc.tile_pool(name="estp", bufs=8))
            smallp = ctx.enter_context(tc.tile_pool(name="smallp", bufs=2))
            stagep = ctx.enter_context(tc.tile_pool(name="stagep", bufs=10))

            ps_big = ctx.enter_context(
                tc.tile_pool(name="ps_big", bufs=2, space="PSUM"))
            ps_sm = ctx.enter_context(
                tc.tile_pool(name="ps_sm", bufs=2, space="PSUM"))
            ps_sum = ctx.enter_context(
                tc.tile_pool(name="ps_sum", bufs=2, space="PSUM"))

            # ---- prologue DMAs ----------------------------------------
            # x0 + wv striped per-ko so the first V group's k-loop can
            # trickle behind the DMA wavefront (dma queues round-robin;
            # strip k lands before strip k+8).  wq follows in quarters so
            # the Q groups can start right after the V groups drain.
            x_tiles = {}
            x0 = xp.tile([P, KO, XCH], BF, tag="xr", name="xr0")
            # single-ko strips for the first 4 ko (earliest possible
            # first matmul), then 2-ko strips (2KB dma lines)
            strip_slices = ([slice(k, k + 1) for k in range(4)]
                            + [slice(k, k + 2) for k in range(4, KO, 2)])
            for ks in strip_slices:
                nc.sync.dma_start(x0[:, ks], xt.ap()[:, ks, 0:XCH])
                nc.sync.dma_start(
                    w_r[:, ks, 2 * NH_LOCAL * P:3 * NH_LOCAL * P],
                    wv.ap()[:, ks])
            x_tiles[0] = x0
            for k2 in range(KO // 2):
                ks = slice(2 * k2, 2 * k2 + 2)
                nc.sync.dma_start(w_r[:, ks, 0:NH_LOCAL * P],
                                  wq.ap()[:, ks])
            for k2 in range(KO // 2):
                ks = slice(2 * k2, 2 * k2 + 2)
                nc.sync.dma_start(
                    w_r[:, ks, NH_LOCAL * P:2 * NH_LOCAL * P],
                    wk.ap()[:, ks])
            nc.sync.dma_start(wo_r[:], wo.ap())

            # ---- phase helpers ----------------------------------------
            def p1_chunk(xc, hooks=None):
                """Project x chunk xc -> Q,K chunk cols + V blocks.
                PSUM->SBUF casts go to the scalar engine (idle here),
                keeping the DVE queue clear for p2/p3 work.  ``hooks``
                are emitted one per group (from the 2nd on) so their
                psum/ACT work rides the projection stream."""
                hooks = list(hooks or ())
                gi = [0]

                def after_group():
                    gi[0] += 1
                    if gi[0] >= 2 and hooks:
                        hooks.pop(0)()

                if xc + 1 < NXCH:     # prefetch next x chunk (4 queues)
                    x_n = xp.tile([P, KO, XCH], BF, tag="xr")
                    for kq in range(4):
                        ks = slice(4 * kq, 4 * (kq + 1))
                        nc.sync.dma_start(
                            x_n[:, ks],
                            xt.ap()[:, ks, (xc + 1) * XCH:(xc + 2) * XCH])
                    x_tiles[xc + 1] = x_n
                x_r = x_tiles[xc]
                # V first (st blocks): out [s=128, d=512]
                for st in range(XCH // P):
                    kb = xc * (XCH // P) + st
                    ps = ps_big.tile([P, 2 * SQ], F32, tag="big")
                    for k in range(KO):
                        nc.tensor.matmul(
                            ps[:, 0:NH_LOCAL * P],
                            x_r[:, k, st * P:(st + 1) * P],
                            w_r[:, k, 2 * NH_LOCAL * P:3 * NH_LOCAL * P],
                            start=(k == 0), stop=(k == KO - 1))
                    nc.scalar.copy(v_sb[:, kb, :], ps[:, 0:NH_LOCAL * P])
                    after_group()
                # Q then K o-tiles: out [o=128, s=XCH]
                for ot in range(2 * NH_LOCAL):
                    ps = ps_big.tile([P, 2 * SQ], F32, tag="big")
                    for k in range(KO):
                        nc.tensor.matmul(
                            ps[:, 0:XCH], w_r[:, k, ot * P:(ot + 1) * P],
                            x_r[:, k], start=(k == 0), stop=(k == KO - 1))
                    nc.scalar.copy(
                        qk_sb[:, ot, xc * XCH:(xc + 1) * XCH], ps[:, 0:XCH])
                    after_group()
                while hooks:
                    hooks.pop(0)()

            def p2_chunk(qc):
                """Causal attention for q chunk qc, all local heads.
                Software-pipelined: consumers (sum+attnV) of pair g are
                emitted two pairs behind scores+exp.  Diagonal blocks are
                processed FIRST so the end-of-head flush lands on pairs
                whose est needs no gpsimd mask; they also stream only
                their valid q range [off, SQ) — the skipped columns are
                exactly the all-masked (zero) ones.  The first processed
                block (kb=4*qc) has off==0, so the start=True matmuls
                initialize every column."""
                nkb = 4 * (qc + 1)
                npair = nkb // 2
                qs = qc * SQ
                # processed block order: 4 diagonal blocks, then the rest
                blocks = list(range(nkb - 4, nkb)) + list(range(0, nkb - 4))
                state = {}            # h -> (at_ps, sm_ps)
                pending = []          # [(h, pi, est)], consumers lag 2

                def consumers(h, pi, est):
                    at_ps, sm_ps = state[h]
                    # both sums back-to-back: one ones-weights load
                    for j in range(2):
                        pos = 2 * pi + j
                        kb = blocks[pos]
                        off = max(0, kb * P - qs)
                        nc.tensor.matmul(sm_ps[0:16, off:SQ], ones16_bf[:],
                                         est[:, j * SQ + off:(j + 1) * SQ],
                                         start=(pos == 0),
                                         stop=(pos == nkb - 1))
                    for j in range(2):
                        pos = 2 * pi + j
                        kb = blocks[pos]
                        off = max(0, kb * P - qs)
                        nc.tensor.matmul(
                            at_ps[:, off:SQ],
                            v_sb[:, kb, h * P:(h + 1) * P],
                            est[:, j * SQ + off:(j + 1) * SQ],
                            start=(pos == 0), stop=(pos == nkb - 1))
                    if pi == npair - 1:   # head tail
                        logs = smallp.tile([1, SQ], F32, tag="logs")
                        nc.scalar.activation(logs[:], sm_ps[0:1, :], AF.Ln)
                        recip = smallp.tile([1, SQ], BF, tag="recip")
                        nc.scalar.activation(recip[:], logs[:], AF.Exp,
                                             scale=-1.0)
                        # broadcast recip to 128 partitions via K=1 matmul
                        # (reuses the sum-tag psum slot; Ln has already
                        # read the old contents by the time this runs)
                        rep_ps = ps_sum.tile([P, SQ], F32, tag="sum",
                                             name=f"rep{qc}_{h}")
                        nc.tensor.matmul(rep_ps[:], onesrow_bf[:], recip[:],
                                         start=True, stop=True)
                        rep_sb = smallp.tile([P, SQ], F32, tag="rep")
                        nc.vector.tensor_copy(rep_sb[:], rep_ps[:])
                        nc.vector.tensor_mul(attnT[:, h, qs:qs + SQ],
                                             at_ps[:], rep_sb[:])
                        del state[h]

                for h in range(NH_LOCAL):
                    state[h] = (ps_sm.tile([P, SQ], F32, tag="small", name=f"at{qc}_{h}"),
                                ps_sum.tile([P, SQ], F32, tag="sum", name=f"sm{qc}_{h}"))
                    for pi in range(npair):
                        b0, b1 = blocks[2 * pi], blocks[2 * pi + 1]
                        st_ps = ps_big.tile([P, 2 * SQ], F32, tag="big")
                        for j, kb in enumerate((b0, b1)):
                            off = max(0, kb * P - qs)
                            nc.tensor.matmul(
                                st_ps[:, j * SQ + off:(j + 1) * SQ],
                                qk_sb[:, NH_LOCAL + h, kb * P:(kb + 1) * P],
                                qk_sb[:, h, qs + off:qs + SQ],
                                start=True, stop=True)
                        est = estp.tile([P, 2 * SQ], BF, tag="est")
                        off0 = max(0, b0 * P - qs)
                        # [SQ, SQ+off1) holds never-written garbage; its
                        # exp output is never read (consumers start at
                        # off1 within block j=1).
                        nc.scalar.activation(est[:, off0:2 * SQ],
                                             st_ps[:, off0:2 * SQ], AF.Exp,
                                             scale=SCALE)
                        for j, kb in enumerate((b0, b1)):
                            off = max(0, kb * P - qs)
                            if kb * P + P - 1 > qs:  # crosses diagonal
                                nc.gpsimd.affine_select(
                                    est[:, j * SQ + off:(j + 1) * SQ],
                                    est[:, j * SQ + off:(j + 1) * SQ],
                                    [[1, SQ - off]],
                                    mybir.AluOpType.is_ge, 0.0,
                                    base=qs + off - kb * P,
                                    channel_multiplier=-1)
                        if len(pending) >= 3:
                            consumers(*pending.pop(0))
                        pending.append((h, pi, est))
                while pending:
                    consumers(*pending.pop(0))

            def p2_prod0(h, pi, prods):
                """qc=0 producer: scores+exp+mask for pair pi of head h
                (all 4 blocks are diagonal; qs=0)."""
                b0, b1 = 2 * pi, 2 * pi + 1
                st_ps = ps_big.tile([P, 2 * SQ], F32, tag="big",
                                    name=f"st0_{h}_{pi}")
                for j, kb in enumerate((b0, b1)):
                    off = kb * P
                    nc.tensor.matmul(
                        st_ps[:, j * SQ + off:(j + 1) * SQ],
                        qk_sb[:, NH_LOCAL + h, kb * P:(kb + 1) * P],
                        qk_sb[:, h, off:SQ],
                        start=True, stop=True)
                est = estp.tile([P, 2 * SQ], BF, tag="est",
                                name=f"est0_{h}_{pi}")
                off0 = b0 * P
                nc.scalar.activation(est[:, off0:2 * SQ],
                                     st_ps[:, off0:2 * SQ], AF.Exp,
                                     scale=SCALE)
                for j, kb in enumerate((b0, b1)):
                    off = kb * P
                    nc.gpsimd.affine_select(
                        est[:, j * SQ + off:(j + 1) * SQ],
                        est[:, j * SQ + off:(j + 1) * SQ],
                        [[1, SQ - off]],
                        mybir.AluOpType.is_ge, 0.0,
                        base=0, channel_multiplier=-1)
                prods.append((h, pi, est))

            def p2_cons0(prods):
                """qc=0 consumers: all ests are ready (exps ran under
                p1(1)'s projection stream)."""
                state0 = {}
                for h, pi, est in prods:
                    if pi == 0:
                        state0[h] = (
                            ps_sm.tile([P, SQ], F32, tag="small",
                                       name=f"at0_{h}"),
                            ps_sum.tile([P, SQ], F32, tag="sum",
                                        name=f"sm0_{h}"))
                    at_ps, sm_ps = state0[h]
                    for j in range(2):
                        pos = 2 * pi + j
                        off = pos * P
                        nc.tensor.matmul(sm_ps[0:16, off:SQ], ones16_bf[:],
                                         est[:, j * SQ + off:(j + 1) * SQ],
                                         start=(pos == 0), stop=(pos == 3))
                    for j in range(2):
                        pos = 2 * pi + j
                        off = pos * P
                        nc.tensor.matmul(
                            at_ps[:, off:SQ],
                            v_sb[:, pos, h * P:(h + 1) * P],
                            est[:, j * SQ + off:(j + 1) * SQ],
                            start=(pos == 0), stop=(pos == 3))
                    if pi == 1:   # head tail
                        logs = smallp.tile([1, SQ], F32, tag="logs",
                                           name=f"logs0_{h}")
                        nc.scalar.activation(logs[:], sm_ps[0:1, :], AF.Ln)
                        recip = smallp.tile([1, SQ], BF, tag="recip",
                                            name=f"recip0_{h}")
                        nc.scalar.activation(recip[:], logs[:], AF.Exp,
                                             scale=-1.0)
                        rep_ps = ps_sum.tile([P, SQ], F32, tag="sum",
                                             name=f"rep0_{h}")
                        nc.tensor.matmul(rep_ps[:], onesrow_bf[:], recip[:],
                                         start=True, stop=True)
                        rep_sb = smallp.tile([P, SQ], F32, tag="rep",
                                             name=f"repsb0_{h}")
                        nc.vector.tensor_copy(rep_sb[:], rep_ps[:])
                        nc.vector.tensor_mul(attnT[:, h, 0:SQ],
                                             at_ps[:], rep_sb[:])

            def p3_chunk(sc):
                """o_proj partial for q chunk sc."""
                for ot in range(KO):
                    ps = ps_sm.tile([P, SQ], F32, tag="small")
                    for kb in range(NH_LOCAL):
                        nc.tensor.matmul(
                            ps[:], wo_r[:, kb, ot * P:(ot + 1) * P],
                            attnT[:, kb, sc * SQ:(sc + 1) * SQ],
                            start=(kb == 0), stop=(kb == NH_LOCAL - 1))
                    stage = stagep.tile([P, SQ], BF, tag="p3stage")
                    nc.vector.tensor_copy(stage[:], ps[:])
                    nc.sync.dma_start(
                        outt.ap()[:, ot, sc * SQ:(sc + 1) * SQ], stage[:])

            # ---- interleaved pipeline ---------------------------------
            # qc=0's producers ride inside p1(1)'s group stream (the thin
            # 2-pair pipeline is ACT-bound stand-alone); its consumers run
            # after with every est ready.  Then the usual interleave.
            p1_chunk(0)
            prods0 = []
            hooks0 = [(lambda h=h, pi=pi: p2_prod0(h, pi, prods0))
                      for h in range(NH_LOCAL) for pi in range(2)]
            p1_chunk(1, hooks=hooks0)
            p2_cons0(prods0)
            p3_chunk(0)
            for xc in range(1, NXCH):
                p2_chunk(xc)
                if xc + 1 < NXCH:
                    p1_chunk(xc + 1)
                p3_chunk(xc)
    _split_multi_waits(nc)
    return nc


_NC_CACHE = None


def _get_nc():
    global _NC_CACHE
    if _NC_CACHE is None:
        _NC_CACHE = build()
    return _NC_CACHE


def _prep_inputs(hidden_states, w_qkv, w_o):
    """Host-side shard + pre-tile + bf16-cast for the 8 cores."""
    import ml_dtypes
    BF_NP = ml_dtypes.bfloat16
    hidden_states = np.asarray(hidden_states, dtype=np.float32)
    w_qkv = np.asarray(w_qkv, dtype=np.float32)
    w_o = np.asarray(w_o, dtype=np.float32)
    B = hidden_states.shape[0]

    in_maps = []
    xt_by_b = {}
    for b in range(B):
        # xt[p, ko, s] = hidden[b, s, ko*128+p]
        xt = np.ascontiguousarray(
            hidden_states[b].T.reshape(KO, P, S).transpose(1, 0, 2)
        ).astype(BF_NP)
        xt_by_b[b] = xt
    for c in range(8):
        b = c // 4
        hs = [4 * (c % 4) + j for j in range(NH_LOCAL)]
        q_rows = np.concatenate([np.arange(h * P, (h + 1) * P) for h in hs])
        k_rows = q_rows + H
        v_rows = q_rows + 2 * H

        def wtile(rows):
            # [p, ko, o] = w_qkv[rows[o], ko*128+p]
            w = w_qkv[rows, :]                      # [512, 2048]
            return np.ascontiguousarray(
                w.T.reshape(KO, P, len(rows)).transpose(1, 0, 2)
            ).astype(BF_NP)

        # wo[p, kb, o] = w_o[o, cols[kb*128+p]]
        wo_c = np.ascontiguousarray(
            w_o[:, q_rows].T.reshape(NH_LOCAL, P, S).transpose(1, 0, 2)
        ).astype(BF_NP)
        in_maps.append({
            "xt": xt_by_b[b],
            "wq": wtile(q_rows),
            "wk": wtile(k_rows),
            "wv": wtile(v_rows),
            "wo": wo_c,
        })
    return in_maps


def run(hidden_states, w_qkv, w_o, trace=False, trace_cores=None):
    in_maps = _prep_inputs(hidden_states, w_qkv, w_o)
    nc = _get_nc()
    kwargs = {}
    if trace:
        kwargs["trace_cores"] = (trace_cores if trace_cores is not None
                                 else list(range(8)))
    res = run_bass_kernel_spmd(nc, in_maps, core_ids=list(range(8)),
                               trace=trace, **kwargs)
    B, S_, H_ = np.asarray(hidden_states).shape
    out = np.zeros((B, S_, H_), dtype=np.float32)
    for c in range(8):
        b = c // 4
        outt = np.asarray(res.results[c]["outt"], dtype=np.float32)
        outT = outt.transpose(1, 0, 2).reshape(H_, S_)   # [128,16,2048]
        out[b] += outT.T
    return out, res


def kernel(hidden_states, w_qkv, w_o):
    out, _ = run(hidden_states, w_qkv, w_o, trace=False)
    return out


# revision 45
# speedup vs baseline: 1.0233x; 1.0008x over previous
"""Tensor-parallel causal attention kernel for 8 trn2 NeuronCores.

Problem: B=2, S=2048, H=2048, 16 heads, head_dim=128 fp32.
  qkv = hidden @ w_qkv.T ; causal attention ; out = attn @ w_o.T

Sharding (hardcoded): core c in 0..7 handles batch b=c//4 and heads
hs = [4*(c%4) .. 4*(c%4)+3].  Each core computes a partial o_proj
output (contraction over its 512 hidden dims); the host sums the 4
partials per batch and transposes.  No device collectives.

Device-side layout (all host-pretiled to partition-major [128, ...]):
  xt  [128,16,2048] f32 : xt[p,ko,s]  = hidden[b, s, ko*128+p]
  wq  [128,16, 512] f32 : wq[p,ko,o]  = w_qkv[q_rows[o],  ko*128+p]
  wk  [128,16, 512] f32 : wk[p,ko,o]  = w_qkv[k_rows[o],  ko*128+p]
  wv  [128,16, 512] f32 : wv[p,ko,d]  = w_qkv[v_rows[d],  ko*128+p]
  wo  [128, 4,2048] f32 : wo[p,kb,o]  = w_o[o, cols[kb*128+p]]
  outt[128,16,2048] f32 : outt[p,ot,s] = outT_partial[ot*128+p, s]

Design (single interleaved pipeline over four 512-token chunks):
  p1(0), p2(0), p1(1), p3(0), p2(1), p1(2), p3(1), ... p2(3), p3(3)
  - p1 projects one x chunk (Q,K columns + V blocks); psum->sbuf casts
    run on the scalar engine (idle during p1), prologue DMAs are
    per-ko-striped so the first V group trickles behind the DMA wave.
  - p2 is software-pipelined causal attention: sum+attnV consumers
    trail scores+exp by three k-pairs (qc=0's producers ride inside
    p1(1)'s group stream since its 2-pair pipeline is ACT-bound
    stand-alone, consumers run after); diagonal blocks are processed
    first and stream only their valid q range (block-128-granular
    causality, bit-identical results); softmax sums use a [P,16] ones
    stationary so the PSUM drain spreads across 16 partitions.
  - p3 o_proj partials stage through bf16 sbuf tiles (10 bufs) and DMA
    out in bf16; the host accumulates the 4 partials per batch in f32.

Toolchain quirk workarounds (walrus 1-sync-wait slot):
  - chunked tail drain monkeypatch (single sem-only barrier, no
    trailing barrier)
  - post-pass hoisting extra sem waits onto same-engine NoOps
"""
import numpy as np

import concourse.bass as bass
import concourse.mybir as mybir
import concourse.tile as tile
from concourse.bass_utils import run_bass_kernel_spmd
from concourse.vector_clock import ScopedClock, VectorClock

P = 128
S = 2048
H = 2048
NH_LOCAL = 4          # heads per core
KO = H // P           # 16 contraction chunks for the projections
SQ = 512              # q chunk width
NQC = S // SQ         # 4 q chunks
NKB = S // P          # 16 key blocks
F32 = mybir.dt.float32
BF = mybir.dt.bfloat16
AF = mybir.ActivationFunctionType
SCALE = 1.0 / float(np.sqrt(128.0))

XCH = SQ              # x chunk width == q chunk width (interleaved pipe)
NXCH = S // XCH


def _drain_and_barrier_chunked(self, tick_clock, wait_clock, _MAX=1):
    """Split the kernel-tail drain's waits: walrus allows only one sync
    wait per instruction in this toolchain."""
    g = tick_clock.global_clock
    n = len(g)
    vals = [g[i] for i in range(n)]
    nz = [i for i, v in enumerate(vals) if v > 0]
    chunks = [nz[i:i + _MAX] for i in range(0, len(nz), _MAX)] or [[]]
    for chunk in chunks:
        vec = [vals[i] if i in chunk else 0 for i in range(n)]
        d = self.nc.sync.drain()
        wait_clock.add_sem_waits(d.ins, ScopedClock({None: VectorClock(vec)}))
    self.nc.all_engine_barrier(sem_only=True)
    assert self.sems is not None
    popped = self.nc._tile_sem_poison_stack.pop()
    assert popped is self._sem_poison
    # no trailing barrier: engines are already synced by the barrier
    # above, and the sem clear below is the true last operation.
    self.nc.clear_and_free_semaphores(list(self.sems.allocated().values()))


tile.TileContext._drain_and_barrier = _drain_and_barrier_chunked


def _split_multi_waits(nc):
    """walrus allows ONE sync wait per instruction: hoist extra waits onto
    same-engine NoOps inserted directly before the offending instruction
    (identical semantics — the engine queue blocks on each in turn)."""
    ctr = 0
    for f in nc.m.functions:
        for blk in f.blocks:
            new = []
            changed = False
            for inst in blk.instructions:
                si = inst.sync_info
                waits = list(si.on_wait) if si and si.on_wait else []
                if len(waits) > 1:
                    changed = True
                    for w in waits[:-1]:
                        ctr += 1
                        nop = mybir.InstNoOp(name=f"I-wsplit-{ctr}",
                                             engine=inst.engine,
                                             ins=[], outs=[])
                        nop.sync_info = mybir.SyncInfo(on_wait=[w],
                                                       on_update=[])
                        new.append(nop)
                    ups = list(si.on_update) if si.on_update else []
                    inst.sync_info = mybir.SyncInfo(on_wait=[waits[-1]],
                                                    on_update=ups)
                new.append(inst)
            if changed:
                blk.instructions = new
    return ctr


def build():
    nc = bass.Bass()
    xt = nc.dram_tensor("xt", [P, KO, S], BF, kind="ExternalInput")
    wq = nc.dram_tensor("wq", [P, KO, NH_LOCAL * P], BF, kind="ExternalInput")
    wk = nc.dram_tensor("wk", [P, KO, NH_LOCAL * P], BF, kind="ExternalInput")
    wv = nc.dram_tensor("wv", [P, KO, NH_LOCAL * P], BF, kind="ExternalInput")
    wo = nc.dram_tensor("wo", [P, NH_LOCAL, S], BF, kind="ExternalInput")
    outt = nc.dram_tensor("outt", [P, KO, S], BF, kind="ExternalOutput")

    with tile.TileContext(nc) as tc:
        from contextlib import ExitStack
        with ExitStack() as ctx:
            const = ctx.enter_context(tc.tile_pool(name="const", bufs=1))
            ones16_bf = const.tile([P, 16], BF)
            nc.vector.memset(ones16_bf[:], 1.0)
            onesrow_bf = const.tile([1, P], BF)
            nc.vector.memset(onesrow_bf[:], 1.0)

            # ---- resident SBUF tensors --------------------------------
            res = ctx.enter_context(tc.tile_pool(name="res", bufs=1))
            # Q,K as qkvT: [d_in, ot(0-3 Q heads, 4-7 K heads), s]
            qk_sb = res.tile([P, 2 * NH_LOCAL, S], BF)
            # V as [s_in, kb, d_local]
            v_sb = res.tile([P, NKB, NH_LOCAL * P], BF)
            attnT = res.tile([P, NH_LOCAL, S], BF)
            w_r = res.tile([P, KO, 3 * NH_LOCAL * P], BF)   # Q|K|V weights
            wo_r = res.tile([P, NH_LOCAL, S], BF)

            xp = ctx.enter_context(tc.tile_pool(name="xp", bufs=2))
            estp = ctx.enter_context(tc.tile_pool(name="estp", bufs=8))
            smallp = ctx.enter_context(tc.tile_pool(name="smallp", bufs=2))
            stagep = ctx.enter_context(tc.tile_pool(name="stagep", bufs=10))

            ps_big = ctx.enter_context(
                tc.tile_pool(name="ps_big", bufs=2, space="PSUM"))
            ps_sm = ctx.enter_context(
                tc.tile_pool(name="ps_sm", bufs=2, space="PSUM"))
            ps_sum = ctx.enter_context(
                tc.tile_pool(name="ps_sum", bufs=2, space="PSUM"))

            # ---- prologue DMAs ----------------------------------------
            # x0 + wv striped per-ko so the first V group's k-loop can
            # trickle behind the DMA wavefront (dma queues round-robin;
            # strip k lands before strip k+8).  wq follows in quarters so
            # the Q groups can start right after the V groups drain.
            x_tiles = {}
            x0 = xp.tile([P, KO, XCH], BF, tag="xr", name="xr0")
            # single-ko strips for the first 4 ko (earliest possible
            # first matmul), then 2-ko strips (2KB dma lines)
            strip_slices = ([slice(k, k + 1) for k in range(4)]
                            + [slice(k, k + 2) for k in range(4, KO, 2)])
            for ks in strip_slices:
                nc.sync.dma_start(x0[:, ks], xt.ap()[:, ks, 0:XCH])
                nc.sync.dma_start(
                    w_r[:, ks, 2 * NH_LOCAL * P:3 * NH_LOCAL * P],
                    wv.ap()[:, ks])
            x_tiles[0] = x0
            for k2 in range(KO // 2):
                ks = slice(2 * k2, 2 * k2 + 2)
                nc.sync.dma_start(w_r[:, ks, 0:NH_LOCAL * P],
                                  wq.ap()[:, ks])
            for k2 in range(KO // 2):
                ks = slice(2 * k2, 2 * k2 + 2)
                nc.sync.dma_start(
                    w_r[:, ks, NH_LOCAL * P:2 * NH_LOCAL * P],
                    wk.ap()[:, ks])
            nc.sync.dma_start(wo_r[:], wo.ap())

            # ---- phase helpers ----------------------------------------
            def p1_chunk(xc, hooks=None):
                """Project x chunk xc -> Q,K chunk cols + V blocks.
                PSUM->SBUF casts go to the scalar engine (idle here),
                keeping the DVE queue clear for p2/p3 work.  ``hooks``
                are emitted one per group (from the 2nd on) so their
                psum/ACT work rides the projection stream."""
                hooks = list(hooks or ())
                gi = [0]

                def after_group():
                    gi[0] += 1
                    if gi[0] >= 2 and hooks:
                        hooks.pop(0)()

                if xc + 1 < NXCH:     # prefetch next x chunk (4 queues)
                    x_n = xp.tile([P, KO, XCH], BF, tag="xr")
                    for kq in range(4):
                        ks = slice(4 * kq, 4 * (kq + 1))
                        nc.sync.dma_start(
                            x_n[:, ks],
                            xt.ap()[:, ks, (xc + 1) * XCH:(xc + 2) * XCH])
                    x_tiles[xc + 1] = x_n
                x_r = x_tiles[xc]
                # V first (st blocks): out [s=128, d=512]
                for st in range(XCH // P):
                    kb = xc * (XCH // P) + st
                    ps = ps_big.tile([P, 2 * SQ], F32, tag="big")
                    for k in range(KO):
                        nc.tensor.matmul(
                            ps[:, 0:NH_LOCAL * P],
                            x_r[:, k, st * P:(st + 1) * P],
                            w_r[:, k, 2 * NH_LOCAL * P:3 * NH_LOCAL * P],
                            start=(k == 0), stop=(k == KO - 1))
                    nc.scalar.copy(v_sb[:, kb, :], ps[:, 0:NH_LOCAL * P])
                    after_group()
                # Q then K o-tiles: out [o=128, s=XCH]
                for ot in range(2 * NH_LOCAL):
                    ps = ps_big.tile([P, 2 * SQ], F32, tag="big")
                    for k in range(KO):
                        nc.tensor.matmul(
                            ps[:, 0:XCH], w_r[:, k, ot * P:(ot + 1) * P],
                            x_r[:, k], start=(k == 0), stop=(k == KO - 1))
                    nc.scalar.copy(
                        qk_sb[:, ot, xc * XCH:(xc + 1) * XCH], ps[:, 0:XCH])
                    after_group()
                while hooks:
                    hooks.pop(0)()

            def p2_chunk(qc):
                """Causal attention for q chunk qc, all local heads.
                Software-pipelined: consumers (sum+attnV) of pair g are
                emitted two pairs behind scores+exp.  Diagonal blocks are
                processed FIRST so the end-of-head flush lands on pairs
                whose est needs no gpsimd mask; they also stream only
                their valid q range [off, SQ) — the skipped columns are
                exactly the all-masked (zero) ones.  The first processed
                block (kb=4*qc) has off==0, so the start=True matmuls
                initialize every column."""
                nkb = 4 * (qc + 1)
                npair = nkb // 2
                qs = qc * SQ
                # processed block order: 4 diagonal blocks, then the rest
                blocks = list(range(nkb - 4, nkb)) + list(range(0, nkb - 4))
                state = {}            # h -> (at_ps, sm_ps)
                pending = []          # [(h, pi, est)], consumers lag 2

                def consumers(h, pi, est):
                    at_ps, sm_ps = state[h]
                    # both sums back-to-back: one ones-weights load
                    for j in range(2):
                        pos = 2 * pi + j
                        kb = blocks[pos]
                        off = max(0, kb * P - qs)
                        nc.tensor.matmul(sm_ps[0:16, off:SQ], ones16_bf[:],
                                         est[:, j * SQ + off:(j + 1) * SQ],
                                         start=(pos == 0),
                                         stop=(pos == nkb - 1))
                    for j in range(2):
                        pos = 2 * pi + j
                        kb = blocks[pos]
                        off = max(0, kb * P - qs)
                        nc.tensor.matmul(
                            at_ps[:, off:SQ],
                            v_sb[:, kb, h * P:(h + 1) * P],
                            est[:, j * SQ + off:(j + 1) * SQ],
                            start=(pos == 0), stop=(pos == nkb - 1))
                    if pi == npair - 1:   # head tail
                        logs = smallp.tile([1, SQ], F32, tag="logs")
                        nc.scalar.activation(logs[:], sm_ps[0:1, :], AF.Ln)
                        recip = smallp.tile([1, SQ], BF, tag="recip")
                        nc.scalar.activation(recip[:], logs[:], AF.Exp,
                                             scale=-1.0)
                        # broadcast recip to 128 partitions via K=1 matmul
                        # (reuses the sum-tag psum slot; Ln has already
                        # read the old contents by the time this runs)
                        rep_ps = ps_sum.tile([P, SQ], F32, tag="sum",
                                             name=f"rep{qc}_{h}")
                        nc.tensor.matmul(rep_ps[:], onesrow_bf[:], recip[:],
                                         start=True, stop=True)
                        rep_sb = smallp.tile([P, SQ], F32, tag="rep")
                        nc.vector.tensor_copy(rep_sb[:], rep_ps[:])
                        nc.vector.tensor_mul(attnT[:, h, qs:qs + SQ],
                                             at_ps[:], rep_sb[:])
                        del state[h]

                for h in range(NH_LOCAL):
                    state[h] = (ps_sm.tile([P, SQ], F32, tag="small", name=f"at{qc}_{h}"),
                                ps_sum.tile([P, SQ], F32, tag="sum", name=f"sm{qc}_{h}"))
                    for pi in range(npair):
                        b0, b1 = blocks[2 * pi], blocks[2 * pi + 1]
                        st_ps = ps_big.tile([P, 2 * SQ], F32, tag="big")
                        for j, kb in enumerate((b0, b1)):
                            off = max(0, kb * P - qs)
                            nc.tensor.matmul(
                                st_ps[:, j * SQ + off:(j + 1) * SQ],
                                qk_sb[:, NH_LOCAL + h, kb * P:(kb + 1) * P],
                                qk_sb[:, h, qs + off:qs + SQ],
                                start=True, stop=True)
                        est = estp.tile([P, 2 * SQ], BF, tag="est")
                        off0 = max(0, b0 * P - qs)
                        # [SQ, SQ+off1) holds never-written garbage; its
                        # exp output is never read (consumers start at
                        # off1 within block j=1).
                        nc.scalar.activation(est[:, off0:2 * SQ],
                                             st_ps[:, off0:2 * SQ], AF.Exp,
                                             scale=SCALE)
                        for j, kb in enumerate((b0, b1)):
                            off = max(0, kb * P - qs)
                            if kb * P + P - 1 > qs:  # crosses diagonal
                                nc.gpsimd.affine_select(
                                    est[:, j * SQ + off:(j + 1) * SQ],
                                    est[:, j * SQ + off:(j + 1) * SQ],
                                    [[1, SQ - off]],
                                    mybir.AluOpType.is_ge, 0.0,
                                    base=qs + off - kb * P,
                                    channel_multiplier=-1)
                        if len(pending) >= 4:
                            consumers(*pending.pop(0))
                        pending.append((h, pi, est))
                while pending:
                    consumers(*pending.pop(0))

            def p2_prod0(h, pi, prods):
                """qc=0 producer: scores+exp+mask for pair pi of head h
                (all 4 blocks are diagonal; qs=0)."""
                b0, b1 = 2 * pi, 2 * pi + 1
                st_ps = ps_big.tile([P, 2 * SQ], F32, tag="big",
                                    name=f"st0_{h}_{pi}")
                for j, kb in enumerate((b0, b1)):
                    off = kb * P
                    nc.tensor.matmul(
                        st_ps[:, j * SQ + off:(j + 1) * SQ],
                        qk_sb[:, NH_LOCAL + h, kb * P:(kb + 1) * P],
                        qk_sb[:, h, off:SQ],
                        start=True, stop=True)
                est = estp.tile([P, 2 * SQ], BF, tag="est",
                                name=f"est0_{h}_{pi}")
                off0 = b0 * P
                nc.scalar.activation(est[:, off0:2 * SQ],
                                     st_ps[:, off0:2 * SQ], AF.Exp,
                                     scale=SCALE)
                for j, kb in enumerate((b0, b1)):
                    off = kb * P
                    nc.gpsimd.affine_select(
                        est[:, j * SQ + off:(j + 1) * SQ],
                        est[:, j * SQ + off:(j + 1) * SQ],
                        [[1, SQ - off]],
                        mybir.AluOpType.is_ge, 0.0,
                        base=0, channel_multiplier=-1)
                prods.append((h, pi, est))

            def p2_cons0(prods):
                """qc=0 consumers: all ests are ready (exps ran under
                p1(1)'s projection stream)."""
                state0 = {}
                for h, pi, est in prods:
                    if pi == 0:
                        state0[h] = (
                            ps_sm.tile([P, SQ], F32, tag="small",
                                       name=f"at0_{h}"),
                            ps_sum.tile([P, SQ], F32, tag="sum",
                                        name=f"sm0_{h}"))
                    at_ps, sm_ps = state0[h]
                    for j in range(2):
                        pos = 2 * pi + j
                        off = pos * P
                        nc.tensor.matmul(sm_ps[0:16, off:SQ], ones16_bf[:],
                                         est[:, j * SQ + off:(j + 1) * SQ],
                                         start=(pos == 0), stop=(pos == 3))
                    for j in range(2):
                        pos = 2 * pi + j
                        off = pos * P
                        nc.tensor.matmul(
                            at_ps[:, off:SQ],
                            v_sb[:, pos, h * P:(h + 1) * P],
                            est[:, j * SQ + off:(j + 1) * SQ],
                            start=(pos == 0), stop=(pos == 3))
                    if pi == 1:   # head tail
                        logs = smallp.tile([1, SQ], F32, tag="logs",
                                           name=f"logs0_{h}")
                        nc.scalar.activation(logs[:], sm_ps[0:1, :], AF.Ln)
                        recip = smallp.tile([1, SQ], BF, tag="recip",
                                            name=f"recip0_{h}")
                        nc.scalar.activation(recip[:], logs[:], AF.Exp,
                                             scale=-1.0)
                        rep_ps = ps_sum.tile([P, SQ], F32, tag="sum",
                                             name=f"rep0_{h}")
                        nc.tensor.matmul(rep_ps[:], onesrow_bf[:], recip[:],
                                         start=True, stop=True)
                        rep_sb = smallp.tile([P, SQ], F32, tag="rep",
                                             name=f"repsb0_{h}")
                        nc.vector.tensor_copy(rep_sb[:], rep_ps[:])
                        nc.vector.tensor_mul(attnT[:, h, 0:SQ],
                                             at_ps[:], rep_sb[:])

            def p3_chunk(sc):
                """o_proj partial for q chunk sc."""
                for ot in range(KO):
                    ps = ps_sm.tile([P, SQ], F32, tag="small")
                    for kb in range(NH_LOCAL):
                        nc.tensor.matmul(
                            ps[:], wo_r[:, kb, ot * P:(ot + 1) * P],
                            attnT[:, kb, sc * SQ:(sc + 1) * SQ],
                            start=(kb == 0), stop=(kb == NH_LOCAL - 1))
                    stage = stagep.tile([P, SQ], BF, tag="p3stage")
                    nc.vector.tensor_copy(stage[:], ps[:])
                    nc.sync.dma_start(
                        outt.ap()[:, ot, sc * SQ:(sc + 1) * SQ], stage[:])

            # ---- interleaved pipeline ---------------------------------
            # qc=0's producers ride inside p1(1)'s group stream (the thin
            # 2-pair pipeline is ACT-bound stand-alone); its consumers run
            # after with every est ready.  Then the usual interleave.
            p1_chunk(0)
            prods0 = []
            hooks0 = [(lambda h=h, pi=pi: p2_prod0(h, pi, prods0))
                      for h in range(NH_LOCAL) for pi in range(2)]
            p1_chunk(1, hooks=hooks0)
            p2_cons0(prods0)
            p3_chunk(0)
            for xc in range(1, NXCH):
                p2_chunk(xc)
                if xc + 1 < NXCH:
                    p1_chunk(xc + 1)
                p3_chunk(xc)
    _split_multi_waits(nc)
    return nc


_NC_CACHE = None


def _get_nc():
    global _NC_CACHE
    if _NC_CACHE is None:
        _NC_CACHE = build()
    return _NC_CACHE


def _prep_inputs(hidden_states, w_qkv, w_o):
    """Host-side shard + pre-tile + bf16-cast for the 8 cores."""
    import ml_dtypes
    BF_NP = ml_dtypes.bfloat16
    hidden_states = np.asarray(hidden_states, dtype=np.float32)
    w_qkv = np.asarray(w_qkv, dtype=np.float32)
    w_o = np.asarray(w_o, dtype=np.float32)
    B = hidden_states.shape[0]

    in_maps = []
    xt_by_b = {}
    for b in range(B):
        # xt[p, ko, s] = hidden[b, s, ko*128+p]
        xt = np.ascontiguousarray(
            hidden_states[b].T.reshape(KO, P, S).transpose(1, 0, 2)
        ).astype(BF_NP)
        xt_by_b[b] = xt
    for c in range(8):
        b = c // 4
        hs = [4 * (c % 4) + j for j in range(NH_LOCAL)]
        q_rows = np.concatenate([np.arange(h * P, (h + 1) * P) for h in hs])
        k_rows = q_rows + H
        v_rows = q_rows + 2 * H

        def wtile(rows):
            # [p, ko, o] = w_qkv[rows[o], ko*128+p]
            w = w_qkv[rows, :]                      # [512, 2048]
            return np.ascontiguousarray(
                w.T.reshape(KO, P, len(rows)).transpose(1, 0, 2)
            ).astype(BF_NP)

        # wo[p, kb, o] = w_o[o, cols[kb*128+p]]
        wo_c = np.ascontiguousarray(
            w_o[:, q_rows].T.reshape(NH_LOCAL, P, S).transpose(1, 0, 2)
        ).astype(BF_NP)
        in_maps.append({
            "xt": xt_by_b[b],
            "wq": wtile(q_rows),
            "wk": wtile(k_rows),
            "wv": wtile(v_rows),
            "wo": wo_c,
        })
    return in_maps


def run(hidden_states, w_qkv, w_o, trace=False, trace_cores=None):
    in_maps = _prep_inputs(hidden_states, w_qkv, w_o)
    nc = _get_nc()
    kwargs = {}
    if trace:
        kwargs["trace_cores"] = (trace_cores if trace_cores is not None
                                 else list(range(8)))
    res = run_bass_kernel_spmd(nc, in_maps, core_ids=list(range(8)),
                               trace=trace, **kwargs)
    B, S_, H_ = np.asarray(hidden_states).shape
    out = np.zeros((B, S_, H_), dtype=np.float32)
    for c in range(8):
        b = c // 4
        outt = np.asarray(res.results[c]["outt"], dtype=np.float32)
        outT = outt.transpose(1, 0, 2).reshape(H_, S_)   # [128,16,2048]
        out[b] += outT.T
    return out, res


def kernel(hidden_states, w_qkv, w_o):
    out, _ = run(hidden_states, w_qkv, w_o, trace=False)
    return out
